# revision 1
# baseline (speedup 1.0000x reference)
"""Multi-head attention block (B=4, N=2048, D=1024, H=16) on 8 trn2 NeuronCores.

Sharding: core c -> (batch b = c//2, head-group g = c%2). Each core computes
attention for 8 heads of one batch plus the partial output projection over its
512 head-dims; the host sums the two partials per batch and adds b_proj.

Per-core kernel (all matmuls in fp32r at free-dim 512 -> full PE rate):
  1. x -> xT via PE transposes (exact: matmul by identity).
  2. qT/kT computed head-transposed ([dims, tokens], lhsT = w slice);
     v computed natural ([tokens, dims], lhsT = xT) with a ones column
     appended per head (v_aug) so the PV matmul also yields the softmax
     denominator (row 64 of the PSUM tile).
  3. S^T tiles [k=128, q=512] for the two heads of a pair computed by two
     row-group-packed matmuls (tile_position rows 0/64) that run
     concurrently on disjoint halves of the PE array (head_dim=64), into
     one 2-bank PSUM tile [128, 1024].
  4. E = exp(scale * S^T) on ScalarE straight out of PSUM, one FD=1024 op
     covering both heads (scores are ~N(0,1): no max subtraction needed).
  5. PV: outT[d,q] accumulated over 16 k-tiles; reciprocal of the
     denominator row is broadcast across partitions by DMA and applied
     on VectorE.
  6. proj: out[t,e] with lhsT = attnT directly; DMA partial to DRAM.
"""

import os
import sys

import numpy as np

try:
    import concourse.bass as bass
except ImportError:  # harness runs from a bare directory
    sys.path.insert(0, "/opt/trn_rl_repo")
    import concourse.bass as bass

import concourse.mybir as mybir
import concourse.tile as tile
from concourse.bass_utils import run_bass_kernel_spmd
from concourse.masks import make_identity

F32 = mybir.dt.float32
F32R = mybir.dt.float32r
EXP = mybir.ActivationFunctionType.Exp
ADD = mybir.AluOpType.add
MULT = mybir.AluOpType.mult

B, N_FULL, D = 4, 2048, 1024
H, HD = 16, 64
NCORES = 8
GROUPS = 2          # head-groups (tensor parallel)
HL = H // GROUPS    # 8 heads per core
DL = HL * HD        # 512 local head-dims per core
PAIRS = HL // 2     # 4 head pairs
SCALE = HD ** -0.5

LAST_EXEC_NS = None


def _split_multiwait_matmuls(raw: bytes) -> bytes:
    """This container's walrus allows at most one sync-wait per Matmult.

    Tile attaches up to 3. Hoist the extras onto standalone EventSemaphore
    instructions inserted immediately before the matmul on the same engine
    (identical semantics: the sequencer blocks on them in program order).
    """
    import json

    bir = json.loads(raw)
    n = [0]

    def fix_block(block):
        insts = block.get("instructions")
        if not isinstance(insts, list):
            return
        out = []
        for ins in insts:
            si = ins.get("sync_info") if isinstance(ins, dict) else None
            if (
                isinstance(ins, dict)
                and ins.get("opcode") != "EventSemaphore"
                and si
                and len(si.get("on_wait") or []) > 1
            ):
                waits = si["on_wait"]
                for w in waits[1:]:
                    n[0] += 1
                    out.append({
                        "debug": ins.get("debug", 0),
                        "engine": ins["engine"],
                        "ins": [],
                        "name": f"I-waitfix-{n[0]}",
                        "opcode": "EventSemaphore",
                        "outs": [],
                        "sync_info": {"on_update": [], "on_wait": [w]},
                    })
                si["on_wait"] = waits[:1]
            out.append(ins)
        block["instructions"] = out

    for fn in bir.get("functions", []):
        for block in fn.get("blocks", []):
            fix_block(block)
    return json.dumps(bir).encode()


def build(N=N_FULL):
    NK = N // 128   # k tiles of 128
    NQ = N // 512   # q tiles of 512
    NTT = N // 512  # token tiles of 512 for the qkv projection

    nc = bass.Bass("TRN2", target_bir_lowering=False)
    x = nc.dram_tensor("x", [N, D], F32, kind="ExternalInput")
    # [ii, otile(4 q-pairs then 4 k-pairs), io, 128] so each DMA slab is
    # contiguous per partition.
    wqk = nc.dram_tensor("wqk", [128, 8, 8, 128], F32R, kind="ExternalInput")
    wv = nc.dram_tensor("wv", [128, 8, DL], F32R, kind="ExternalInput")
    bqk = nc.dram_tensor("bqk", [128, 8], F32, kind="ExternalInput")
    bv = nc.dram_tensor("bv", [128, DL], F32, kind="ExternalInput")
    wproj = nc.dram_tensor("wproj", [128, PAIRS, D], F32R, kind="ExternalInput")
    out = nc.dram_tensor("out", [N, D], F32, kind="ExternalOutput")

    with tile.TileContext(nc) as tc:
        with (
            tc.tile_pool(name="const", bufs=1) as const_pool,
            tc.tile_pool(name="wres", bufs=1) as wres_pool,
            tc.tile_pool(name="wqs", bufs=2) as wqs_pool,
            tc.tile_pool(name="xn", bufs=2) as xn_pool,
            tc.tile_pool(name="xt", bufs=1) as xt_pool,
            tc.tile_pool(name="qk", bufs=1) as qk_pool,
            tc.tile_pool(name="vg", bufs=1) as vg_pool,
            tc.tile_pool(name="at", bufs=2) as at_pool,
            tc.tile_pool(name="ep", bufs=3) as e_pool,
            tc.tile_pool(name="rp", bufs=3) as r_pool,
            tc.tile_pool(name="rb", bufs=3) as rb_pool,
            tc.tile_pool(name="ob", bufs=2) as ob_pool,
            tc.tile_pool(name="psst", bufs=2, space="PSUM") as pss_pool,
            tc.tile_pool(name="pspv", bufs=4, space="PSUM") as psv_pool,
            tc.tile_pool(name="dr", bufs=2, space="DRAM") as dr_pool,
        ):
            ident = const_pool.tile([128, 128], F32)
            make_identity(nc, ident[:, :])
            bqk_sb = const_pool.tile([128, 8], F32)
            nc.sync.dma_start(bqk_sb[:, :], bqk[:, :])

            qT = qk_pool.tile([128, PAIRS, N], F32R, tag="qT")
            kT = qk_pool.tile([128, PAIRS, N], F32R, tag="kT")
            # Flat v layout: per (k-tile, head) a 65-column group = 64 v-dims
            # + ones column (PV denominator row). +63 tail pad so every PV
            # lhsT can read a full 32-aligned M=128 window (the ISA rejects
            # M=65 dst partitions; the over-read rows land in psum rows
            # 65:127 and are never read). Matmul time is N-cycles, so the
            # padding is free.
            VG = HD + 1
            vaug = vg_pool.tile([128, NK * HL * VG + 128 - VG], F32R, tag="vaug")
            ones_view = vaug[:, 0:NK * HL * VG].rearrange(
                "p (g c) -> p g c", c=VG)[:, :, HD:HD + 1]
            nc.vector.tensor_scalar(
                out=ones_view, in0=bqk_sb[:, None, 0:1].broadcast_to(
                    [128, NK * HL, 1]),
                scalar1=0.0, scalar2=1.0, op0=MULT, op1=ADD,
            )
            # tail pad (finite filler so the last PV over-read is defined)
            nc.vector.tensor_scalar(
                out=vaug[:, NK * HL * VG:],
                in0=bqk_sb[:, 0:1].broadcast_to([128, 128 - VG]),
                scalar1=0.0, scalar2=1.0, op0=MULT, op1=ADD,
            )

            def attn_kt(pvA, pvB, p, qn, kt):
                q0 = qn * 512
                k0 = kt * 128
                stab = pss_pool.tile([128, 1024], F32, tag="st", name="stab")
                for fo, base in ((0, 0), (512, 64)):
                    nc.tensor.matmul(
                        stab[:, fo:fo + 512],
                        lhsT=kT[base:base + 64, p, k0:k0 + 128],
                        rhs=qT[base:base + 64, p, q0:q0 + 512],
                        start=True,
                        stop=True,
                        tile_position=(base, 0),
                        skip_group_check=True,
                    )
                e2 = e_pool.tile([128, 1024], F32R, tag="e", name="e2")
                nc.scalar.activation(e2[:, :], stab[:, :], EXP, scale=SCALE)
                for pv, hh in ((pvA, 0), (pvB, 1)):
                    vo = (kt * HL + 2 * p + hh) * VG
                    nc.tensor.matmul(
                        pv[:, :],
                        lhsT=vaug[:, vo:vo + 128],
                        rhs=e2[:, hh * 512:(hh + 1) * 512],
                        start=(kt == 0),
                        stop=(kt == NK - 1),
                        skip_group_check=True,
                    )

            def attn_norm(pvA, pvB, at_t, p):
                for hh, pv in ((0, pvA), (1, pvB)):
                    rc = r_pool.tile([1, 512], F32, tag="rc", name="rc")
                    nc.vector.reciprocal(rc[:, :], pv[HD:HD + 1, :])
                    rcd = dr_pool.tile([512], F32, tag="rcd", name="rcd")
                    nc.sync.dma_start(rcd[:], rc[0:1, :])
                    rb = rb_pool.tile([64, 512], F32, tag="rb", name="rb")
                    nc.sync.dma_start(rb[:, :], rcd[None, :].broadcast_to([64, 512]))
                    nc.vector.tensor_tensor(
                        out=at_t[hh * 64:(hh + 1) * 64, p, :],
                        in0=pv[0:64, :],
                        in1=rb[:, :],
                        op=MULT,
                    )

            # Early chain: (pair 0, qn 0) runs during the qkv phase — its
            # k-tiles become valid t-tile by t-tile, so its exps fill the
            # otherwise ACT-idle prefix. Holds 2 of the 4 psv slots; qkv's
            # vp/qp rotate through the remaining 2.
            pv0A = psv_pool.tile([128, 512], F32, tag="pv", name="pv0A")
            pv0B = psv_pool.tile([128, 512], F32, tag="pv", name="pv0B")
            at0 = at_pool.tile([128, PAIRS, 512], F32R, tag="at", name="at0")

            # ---- qkv projection (and x transpose), one 512-token tile at a time
            for ti in range(NTT):
                xt = xt_pool.tile([128, 8, 512], F32R, tag="xt")
                for s in range(4):
                    r = ti * 4 + s
                    xn = xn_pool.tile([128, D], F32, tag="xn")
                    nc.sync.dma_start(xn[:, :], x[r * 128:(r + 1) * 128, :])
                    for ic in range(8):
                        tp = pss_pool.tile([128, 128], F32, tag="st")
                        nc.tensor.transpose(
                            tp[:, :], xn[:, ic * 128:(ic + 1) * 128], ident[:, :]
                        )
                        nc.vector.tensor_copy(xt[:, ic, s * 128:(s + 1) * 128], tp[:, :])
                if ti == 0:
                    bv_sb = const_pool.tile([128, DL], F32)
                    nc.sync.dma_start(bv_sb[:, :], bv[:, :])
                    wv_sb = wres_pool.tile([128, 8, DL], F32R)
                    nc.sync.dma_start(wv_sb[:, :, :], wv[:, :, :])
                for s in range(4):
                    r = ti * 4 + s
                    vp = psv_pool.tile([128, DL], F32, tag="pv")
                    for ic in range(8):
                        nc.tensor.matmul(
                            vp[:, :],
                            lhsT=xt[:, ic, s * 128:(s + 1) * 128],
                            rhs=wv_sb[:, ic, :],
                            start=(ic == 0),
                            stop=(ic == 7),
                        )
                    nc.vector.tensor_tensor(
                        out=vaug[:, r * HL * VG:(r + 1) * HL * VG].rearrange(
                            "p (h c) -> p h c", c=VG)[:, :, 0:HD],
                        in0=vp[:, :].rearrange("p (h d) -> p h d", h=HL),
                        in1=bv_sb[:, :].rearrange("p (h d) -> p h d", h=HL),
                        op=ADD,
                    )
                for o in range(8):
                    wo = wqs_pool.tile([128, 8, 128], F32R, tag="wo")
                    nc.sync.dma_start(wo[:, :, :], wqk[:, o, :, :])
                    qp = psv_pool.tile([128, 512], F32, tag="pv")
                    for ic in range(8):
                        nc.tensor.matmul(
                            qp[:, :],
                            lhsT=wo[:, ic, :],
                            rhs=xt[:, ic, :],
                            start=(ic == 0),
                            stop=(ic == 7),
                        )
                    dst = qT if o < 4 else kT
                    nc.vector.tensor_scalar_add(
                        dst[:, o % 4, ti * 512:(ti + 1) * 512], qp[:, :],
                        bqk_sb[:, o:o + 1],
                    )
                for kt in range(ti * 4, ti * 4 + 4):
                    attn_kt(pv0A, pv0B, 0, 0, kt)

            # w_proj is first read by the projection, deep into the
            # attention phase; loading it here keeps the head-of-queue DMA
            # slots for the x tiles the transposes are waiting on.
            wp_sb = wres_pool.tile([128, PAIRS, D], F32R)
            nc.sync.dma_start(wp_sb[:, :, :], wproj[:, :, :])

            def proj(at_t, qn_t):
                for s in range(4):
                    t0 = qn_t * 512 + s * 128
                    for e in range(2):
                        op_ = psv_pool.tile([128, 512], F32, tag="pv")
                        for p_ in range(PAIRS):
                            nc.tensor.matmul(
                                op_[:, :],
                                lhsT=at_t[:, p_, s * 128:(s + 1) * 128],
                                rhs=wp_sb[:, p_, e * 512:(e + 1) * 512],
                                start=(p_ == 0),
                                stop=(p_ == PAIRS - 1),
                            )
                        ob = ob_pool.tile([128, 512], F32, tag="ob")
                        nc.vector.tensor_copy(ob[:, :], op_[:, :])
                        nc.sync.dma_start(
                            out[t0:t0 + 128, e * 512:(e + 1) * 512], ob[:, :])

            # ---- attention + projection, one 512-query tile at a time.
            # proj(qn-1) is emitted after the first pair of qn so the PE
            # work it adds lands inside the ACT-bound stretch of the next
            # attention block instead of stalling ACT at the boundary.
            attn_norm(pv0A, pv0B, at0, 0)
            at_prev = None
            for qn in range(NQ):
                at = at0 if qn == 0 else at_pool.tile(
                    [128, PAIRS, 512], F32R, tag="at", name="at")
                for p in range(PAIRS):
                    if qn == 0 and p == 0:
                        continue  # computed during the qkv phase
                    pvA = psv_pool.tile([128, 512], F32, tag="pv", name="pvA")
                    pvB = psv_pool.tile([128, 512], F32, tag="pv", name="pvB")
                    for kt in range(NK):
                        attn_kt(pvA, pvB, p, qn, kt)
                    attn_norm(pvA, pvB, at, p)
                    if p == 1 and at_prev is not None:
                        proj(at_prev, qn - 1)
                at_prev = at
            proj(at_prev, NQ - 1)
    _orig_to_json = nc.to_json_bytes
    nc.to_json_bytes = lambda: _split_multiwait_matmuls(_orig_to_json())
    return nc


def shard_inputs(x, w_qkv, b_qkv, w_proj, N=N_FULL):
    """Build the 8 per-core input maps from full inputs."""
    x = np.ascontiguousarray(np.asarray(x, dtype=np.float32))
    w_qkv = np.asarray(w_qkv, dtype=np.float32)
    b_qkv = np.asarray(b_qkv, dtype=np.float32)
    w_proj = np.asarray(w_proj, dtype=np.float32)
    in_maps = []
    for c in range(NCORES):
        b, g = divmod(c, 2)
        qc = slice(g * DL, (g + 1) * DL)
        wq = w_qkv[:, 0 * D:1 * D][:, qc]
        wk = w_qkv[:, 1 * D:2 * D][:, qc]
        wv_ = w_qkv[:, 2 * D:3 * D][:, qc]
        wqk_np = np.empty((128, 8, 8, 128), np.float32)
        bqk_np = np.empty((128, 8), np.float32)
        for o in range(8):
            src = wq if o < 4 else wk
            bsrc = b_qkv[0:D][qc] if o < 4 else b_qkv[D:2 * D][qc]
            blk = src[:, (o % 4) * 128:(o % 4 + 1) * 128].reshape(8, 128, 128)
            wqk_np[:, o] = blk.transpose(1, 0, 2)
            bqk_np[:, o] = bsrc[(o % 4) * 128:(o % 4 + 1) * 128]
        wv_np = np.ascontiguousarray(wv_.reshape(8, 128, DL).transpose(1, 0, 2))
        bv_np = np.broadcast_to(b_qkv[2 * D:3 * D][qc], (128, DL)).copy()
        wp_np = np.ascontiguousarray(
            w_proj[g * DL:(g + 1) * DL, :].reshape(PAIRS, 128, D).transpose(1, 0, 2)
        )
        in_maps.append({
            "x": np.ascontiguousarray(x[min(b, x.shape[0] - 1), :N]) if x.ndim == 3
                 else np.ascontiguousarray(x[:N]),
            "wqk": wqk_np,
            "wv": wv_np,
            "bqk": bqk_np,
            "bv": bv_np,
            "wproj": wp_np,
        })
    return in_maps


_NC_CACHE = {}


def kernel(x, w_qkv, b_qkv, w_proj, b_proj):
    global LAST_EXEC_NS
    x = np.asarray(x, dtype=np.float32)
    b_proj = np.asarray(b_proj, dtype=np.float32)
    if N_FULL not in _NC_CACHE:
        _NC_CACHE[N_FULL] = build(N_FULL)
    nc = _NC_CACHE[N_FULL]
    in_maps = shard_inputs(x, w_qkv, b_qkv, w_proj)
    trace = os.environ.get("KERNEL_TRACE", "0") == "1"
    res = run_bass_kernel_spmd(
        nc, in_maps, core_ids=list(range(NCORES)), trace=trace,
        trace_cores=[0] if trace else None,
    )
    LAST_EXEC_NS = res.exec_time_ns
    outs = [r["out"] for r in res.results]
    full = np.empty((B, N_FULL, D), np.float32)
    for b in range(B):
        full[b] = outs[2 * b] + outs[2 * b + 1]
    full += b_proj[None, None, :]
    return full



# revision 4
# speedup vs baseline: 1.2560x; 1.2560x over previous
"""Multi-head attention block (B=4, N=2048, D=1024, H=16) on 8 trn2 NeuronCores.

Sharding: core c -> (batch b = c//2, head-group g = c%2). Each core computes
attention for 8 heads of one batch plus the partial output projection over its
512 head-dims; the host sums the two partials per batch and adds b_proj.

Per-core kernel (fp16 data path, fp32 PSUM accumulation):
  1. x is cast to fp16 on the host; xT tiles arrive via XBAR DMA transpose
     straight from DRAM (no PE transposes, no PSUM->SBUF copies).
  2. qT/kT computed head-transposed ([dims, tokens], lhsT = w slice),
     bias-added into fp16; v computed natural ([tokens, dims]) with a ones
     column per (k-tile, head) group (v_aug) so the flipped PV matmul also
     yields the softmax denominator.
  3. S^T tiles [k=128, q=512] for the two heads of a pair via two
     row-group-packed matmuls into one 2-bank PSUM tile [128, 1024].
  4. E = exp(scale * S^T) on ScalarE straight out of PSUM into fp16.
  5. PV flipped: out[q, d] per (head, 128-query block): lhsT = E slice
     [128k, 128q], rhs = v_aug slice [128k, 65]; 65-column matmuls
     accumulate over k-tiles. PV for k-tile j is emitted after S^T of
     k-tile j+1 so the in-order PE queue never stalls on the exp.
  6. Tail: 2 reciprocals + 2 broadcast multiplies normalize into fp16 u
     tiles; XBAR DMA transpose writes them back as [dims, tokens] for the
     fp16 projection.
"""

import os
import sys

import numpy as np

try:
    import concourse.bass as bass
except ImportError:  # harness runs from a bare directory
    sys.path.insert(0, "/opt/trn_rl_repo")
    import concourse.bass as bass

import concourse.mybir as mybir
import concourse.tile as tile
from concourse.bass_utils import run_bass_kernel_spmd
from concourse.masks import make_identity

F32 = mybir.dt.float32
F16 = mybir.dt.float16
EXP = mybir.ActivationFunctionType.Exp
ADD = mybir.AluOpType.add
MULT = mybir.AluOpType.mult

B, N_FULL, D = 4, 2048, 1024
H, HD = 16, 64
NCORES = 8
GROUPS = 2          # head-groups (tensor parallel)
HL = H // GROUPS    # 8 heads per core
DL = HL * HD        # 512 local head-dims per core
PAIRS = HL // 2     # 4 head pairs
SCALE = HD ** -0.5
VG = HD + 1         # v dims + ones column per (k-tile, head)

LAST_EXEC_NS = None
EXPS = []  # debug: (p, qn, kt) per emitted exp, in ACT-stream order


def _split_multiwait_matmuls(raw: bytes) -> bytes:
    """This container's walrus allows at most one sync-wait per Matmult.

    Tile attaches up to 3. Hoist the extras onto standalone EventSemaphore
    instructions inserted immediately before the matmul on the same engine
    (identical semantics: the sequencer blocks on them in program order).
    """
    import json

    bir = json.loads(raw)
    n = [0]

    def fix_block(block):
        insts = block.get("instructions")
        if not isinstance(insts, list):
            return
        out = []
        for ins in insts:
            si = ins.get("sync_info") if isinstance(ins, dict) else None
            if (
                isinstance(ins, dict)
                and ins.get("opcode") != "EventSemaphore"
                and si
                and len(si.get("on_wait") or []) > 1
            ):
                waits = si["on_wait"]
                for w in waits[1:]:
                    n[0] += 1
                    out.append({
                        "debug": ins.get("debug", 0),
                        "engine": ins["engine"],
                        "ins": [],
                        "name": f"I-waitfix-{n[0]}",
                        "opcode": "EventSemaphore",
                        "outs": [],
                        "sync_info": {"on_update": [], "on_wait": [w]},
                    })
                si["on_wait"] = waits[:1]
            out.append(ins)
        block["instructions"] = out

    for fn in bir.get("functions", []):
        for block in fn.get("blocks", []):
            fix_block(block)
    return json.dumps(bir).encode()


def build(N=N_FULL):
    NK = N // 128   # k tiles of 128
    NQ = N // 512   # q tiles of 512
    NTT = N // 512  # token tiles of 512 for the qkv projection

    nc = bass.Bass("TRN2", target_bir_lowering=False)
    x = nc.dram_tensor("x", [N, D], F16, kind="ExternalInput")
    # [ii, otile(4 q-pairs then 4 k-pairs), io, 128] so each DMA slab is
    # contiguous per partition.
    wqk = nc.dram_tensor("wqk", [128, 8, 8, 128], F16, kind="ExternalInput")
    wv = nc.dram_tensor("wv", [128, 8, DL], F16, kind="ExternalInput")
    bqk = nc.dram_tensor("bqk", [128, 8], F32, kind="ExternalInput")
    bv = nc.dram_tensor("bv", [128, DL], F32, kind="ExternalInput")
    wproj = nc.dram_tensor("wproj", [128, PAIRS, D], F16, kind="ExternalInput")
    out = nc.dram_tensor("out", [N, D], F32, kind="ExternalOutput")

    with tile.TileContext(nc) as tc:
        with (
            tc.tile_pool(name="const", bufs=1) as const_pool,
            tc.tile_pool(name="wres", bufs=1) as wres_pool,
            tc.tile_pool(name="wqs", bufs=2) as wqs_pool,
            tc.tile_pool(name="xt", bufs=4) as xt_pool,
            tc.tile_pool(name="qk", bufs=1) as qk_pool,
            tc.tile_pool(name="vg", bufs=1) as vg_pool,
            tc.tile_pool(name="at", bufs=2) as at_pool,
            tc.tile_pool(name="ep", bufs=3) as e_pool,
            tc.tile_pool(name="rp", bufs=2) as r_pool,
            tc.tile_pool(name="up", bufs=4) as u_pool,
            tc.tile_pool(name="sg", bufs=10) as sg_pool,
            tc.tile_pool(name="ob", bufs=2) as ob_pool,
            tc.tile_pool(name="psst", bufs=2, space="PSUM") as pss_pool,
            tc.tile_pool(name="pspv", bufs=2, space="PSUM") as psv_pool,
            tc.tile_pool(name="ps512", bufs=2, space="PSUM") as p5_pool,
        ):
            # Warm up the ACT exp table while the first DMAs are in flight so
            # the first real exp doesn't pay the table load.
            warm = const_pool.tile([128, 1], F32)
            nc.gpsimd.memset(warm[:, :], 0.0)
            nc.scalar.activation(warm[:, :], warm[:, :], EXP)

            bqk_sb = const_pool.tile([128, 8], F32)

            # fp16 identity for the PE transposes in the attention tail
            # (fp16 memset is ISA-invalid, so build in f32 and cast)
            ident32 = const_pool.tile([128, 128], F32)
            make_identity(nc, ident32[:, :])
            ident16 = const_pool.tile([128, 128], F16)
            nc.vector.tensor_copy(ident16[:, :], ident32[:, :])

            qT = qk_pool.tile([128, PAIRS, N], F16, tag="qT")
            kT = qk_pool.tile([128, PAIRS, N], F16, tag="kT")
            # Flat v layout: per (k-tile, head) a 65-column group = 64 v-dims
            # + ones column (PV denominator column after the flip).
            vaug = vg_pool.tile([128, NK * HL * VG], F16, tag="vaug")
            ones_view = vaug[:, :].rearrange(
                "p (g c) -> p g c", c=VG)[:, :, HD:HD + 1]
            nc.vector.tensor_scalar(
                out=ones_view, in0=warm[:, None, 0:1].broadcast_to(
                    [128, NK * HL, 1]),
                scalar1=0.0, scalar2=1.0, op0=MULT, op1=ADD,
            )

            class Chain:
                """One (pair, qn) attention chain, PV pipelined one kt back.

                Can be spilled mid-way: the PSUM partials move to SBUF
                segment tiles so another chain can use the PSUM banks, and
                segments are merged back in finish().
                """

                def __init__(self, p, qn):
                    self.p, self.qn = p, qn
                    self.pvA = self.pvB = None
                    self.segs = [None, None]
                    self.prev = None   # (e2, kt) awaiting its PV
                    self.first = True

                def _pvs(self):
                    return ((0, self.pvA), (1, self.pvB))

                def _pv(self):
                    e2, kt = self.prev
                    self.prev = None
                    for hh, pv in self._pvs():
                        vo = (kt * HL + 2 * self.p + hh) * VG
                        for qs in range(4):
                            nc.tensor.matmul(
                                pv[:, qs, :],
                                lhsT=e2[:, hh * 512 + qs * 128:
                                        hh * 512 + (qs + 1) * 128],
                                rhs=vaug[:, vo:vo + VG],
                                start=(self.first and qs == 0),
                                stop=False,
                                skip_group_check=True,
                            )
                        if hh == 1:
                            self.first = False

                def step(self, kt):
                    if self.pvA is None:
                        self.pvA = psv_pool.tile([128, 4, VG], F32, tag="pv",
                                                 name="pvA")
                        self.pvB = psv_pool.tile([128, 4, VG], F32, tag="pv",
                                                 name="pvB")
                        self.first = True
                    q0 = self.qn * 512
                    k0 = kt * 128
                    stab = pss_pool.tile([128, 1024], F32, tag="st",
                                         name="stab")
                    for fo, base in ((0, 0), (512, 64)):
                        nc.tensor.matmul(
                            stab[:, fo:fo + 512],
                            lhsT=kT[base:base + 64, self.p, k0:k0 + 128],
                            rhs=qT[base:base + 64, self.p, q0:q0 + 512],
                            start=True,
                            stop=True,
                            tile_position=(base, 0),
                            skip_group_check=True,
                        )
                    e2 = e_pool.tile([128, 1024], F16, tag="e", name="e2")
                    nc.scalar.activation(e2[:, :], stab[:, :], EXP, scale=SCALE)
                    EXPS.append((self.p, self.qn, kt))
                    if self.prev is not None:
                        self._pv()
                    self.prev = (e2, kt)

                def spill(self):
                    """Drain the pending PV and move partials to SBUF."""
                    if self.prev is not None:
                        self._pv()
                    for hh, pv in self._pvs():
                        if self.segs[hh] is None:
                            seg = sg_pool.tile([128, 4, VG], F32, tag="sg",
                                               name="seg")
                            nc.vector.tensor_copy(seg[:, :, :], pv[:, :, :])
                        else:
                            seg = sg_pool.tile([128, 4, VG], F32, tag="sg",
                                               name="seg")
                            nc.vector.tensor_tensor(
                                out=seg[:, :, :], in0=pv[:, :, :],
                                in1=self.segs[hh][:, :, :], op=ADD)
                        self.segs[hh] = seg
                    self.pvA = self.pvB = None

                def finish(self, at_t):
                    if self.prev is not None:
                        self._pv()
                    p = self.p
                    # merge spilled segments, then normalize by the
                    # per-query denominator (column 64) during the fp16 copy
                    rcs = r_pool.tile([128, 2, 4, 1], F32, tag="rc", name="rcs")
                    srcs = []
                    for hh, pv in self._pvs():
                        if self.segs[hh] is not None:
                            fin = sg_pool.tile([128, 4, VG], F32, tag="sg",
                                               name="fin")
                            nc.vector.tensor_tensor(
                                out=fin[:, :, :], in0=pv[:, :, :],
                                in1=self.segs[hh][:, :, :], op=ADD)
                            srcs.append(fin)
                        else:
                            srcs.append(pv)
                    for hh, src in enumerate(srcs):
                        nc.vector.reciprocal(
                            rcs[:, hh, :, :], src[:, :, HD:HD + 1])
                    tr = p5_pool.tile([128, 512], F32, tag="p512", name="tr")
                    for hh, src in enumerate(srcs):
                        u = u_pool.tile([128, 4, HD], F16, tag="u", name="u")
                        nc.vector.tensor_tensor(
                            out=u[:, :, :],
                            in0=src[:, :, 0:HD],
                            in1=rcs[:, hh, :, :].broadcast_to([128, 4, HD]),
                            op=MULT,
                        )
                        for qs in range(4):
                            nc.tensor.matmul(
                                tr[hh * 64:(hh + 1) * 64,
                                   qs * 128:(qs + 1) * 128],
                                lhsT=u[:, qs, :],
                                rhs=ident16[:, :],
                                start=True,
                                stop=True,
                                skip_group_check=True,
                            )
                    nc.vector.tensor_copy(at_t[:, p, :], tr[:, :])
                    self.pvA = self.pvB = None
                    self.segs = [None, None]

            # Early chain: (pair 0, qn 0) runs during the kv phase — its
            # k-tiles become valid t-tile by t-tile, so its exps fill the
            # otherwise ACT-idle prefix.
            at0 = at_pool.tile([128, PAIRS, 512], F16, tag="at", name="at0")

            xts = []

            def qk_half(ti, o, state, half, xt):
                """One half (4 ic) of a q/k projection block, for side-work
                pumping: keeps per-pump PE cost at ~0.85us."""
                if half == 0:
                    wo = wqs_pool.tile([128, 8, 128], F16, tag="wo")
                    nc.sync.dma_start(wo[:, :, :], wqk[:, o, :, :])
                    qp = p5_pool.tile([128, 512], F32, tag="p512", name="qp")
                    state[:] = [wo, qp]
                wo, qp = state
                for ic in range(half * 4, half * 4 + 4):
                    nc.tensor.matmul(
                        qp[:, :],
                        lhsT=wo[:, ic, :],
                        rhs=xt[:, ic, :],
                        start=(ic == 0),
                        stop=(ic == 7),
                    )
                if half == 1:
                    dst = qT if o < 4 else kT
                    nc.vector.tensor_scalar_add(
                        dst[:, o % 4, ti * 512:(ti + 1) * 512], qp[:, :],
                        bqk_sb[:, o:o + 1],
                    )

            def qk_side(ti, o):
                state = []
                return [lambda h=h: qk_half(ti, o, state, h, xts[ti])
                        for h in (0, 1)]

            def qk_group(ti, o, xt):
                """One 128-dim output block of the q/k projection."""
                wo = wqs_pool.tile([128, 8, 128], F16, tag="wo")
                nc.sync.dma_start(wo[:, :, :], wqk[:, o, :, :])
                qp = p5_pool.tile([128, 512], F32, tag="p512", name="qp")
                for ic in range(8):
                    nc.tensor.matmul(
                        qp[:, :],
                        lhsT=wo[:, ic, :],
                        rhs=xt[:, ic, :],
                        start=(ic == 0),
                        stop=(ic == 7),
                    )
                dst = qT if o < 4 else kT
                nc.vector.tensor_scalar_add(
                    dst[:, o % 4, ti * 512:(ti + 1) * 512], qp[:, :],
                    bqk_sb[:, o:o + 1],
                )

            # ---- phase 1: xT (DMA transpose from DRAM), v, kT for every
            # token tile, plus the qn0 queries; q projections for qn1..3 are
            # deferred into the ACT-bound attention stretch (JIT q). The qn0
            # chains run here against the PE-dense stretch, spilling their
            # PSUM partials so only one chain holds banks at a time.
            chains0 = [Chain(p, 0) for p in range(PAIRS)]
            chA, chB, chC, chD = chains0

            def v_group(ti, s, xt):
                r = ti * 4 + s
                vp = p5_pool.tile([128, DL], F32, tag="p512", name="vp")
                for ic in range(8):
                    nc.tensor.matmul(
                        vp[:, :],
                        lhsT=xt[:, ic, s * 128:(s + 1) * 128],
                        rhs=wv_sb[:, ic, :],
                        start=(ic == 0),
                        stop=(ic == 7),
                    )
                nc.vector.tensor_tensor(
                    out=vaug[:, r * HL * VG:(r + 1) * HL * VG].rearrange(
                        "p (h c) -> p h c", c=VG)[:, :, 0:HD],
                    in0=vp[:, :].rearrange("p (h d) -> p h d", h=HL),
                    in1=bv_sb[:, :].rearrange("p (h d) -> p h d", h=HL),
                    op=ADD,
                )

            for ti in range(NTT):
                xt = xt_pool.tile([128, 8, 512], F16, tag="xt",
                                  name=f"xt{ti}")
                xts.append(xt)
                nc.sync.dma_start_transpose(
                    xt[:, :, :], x[ti * 512:(ti + 1) * 512, :])
                if ti == 0:
                    nc.sync.dma_start(bqk_sb[:, :], bqk[:, :])
                    # queries/keys for pair 0 first so the exp stream starts
                    # as early as possible; chain B follows A inside ti0.
                    qk_group(0, 4, xt)
                    qk_group(0, 0, xt)
                    bv_sb = const_pool.tile([128, DL], F32)
                    nc.sync.dma_start(bv_sb[:, :], bv[:, :])
                    wv_sb = wres_pool.tile([128, 8, DL], F16)
                    nc.sync.dma_start(wv_sb[:, :, :], wv[:, :, :])
                    chA.step(0)
                    qk_group(0, 5, xt)
                    qk_group(0, 1, xt)
                    v_group(0, 0, xt)
                    chA.step(1)
                    v_group(0, 1, xt)
                    chA.step(2)
                    v_group(0, 2, xt)
                    chA.step(3)
                    v_group(0, 3, xt)
                    chB.step(0)
                    chA.spill()
                    chB.step(1)
                    chB.step(2)
                    chB.step(3)
                    chB.spill()
                    for o in (6, 2, 7, 3):
                        qk_group(0, o, xt)
                elif ti == 1:
                    qk_group(1, 6, xt)
                    chC.step(0)
                    chC.step(1)
                    v_group(1, 0, xt)
                    chC.step(2)
                    v_group(1, 1, xt)
                    chC.step(3)
                    v_group(1, 2, xt)
                    chC.step(4)
                    v_group(1, 3, xt)
                    chC.step(5)
                    chC.step(6)
                    chC.step(7)
                    qk_group(1, 7, xt)
                    chD.step(0)
                    chC.spill()
                    for kt in range(1, 8):
                        chD.step(kt)
                    chD.spill()
                    for o in (4, 5):
                        qk_group(1, o, xt)
                elif ti == 2:
                    qk_group(2, 4, xt)
                    chA.step(4)
                    chA.step(5)
                    v_group(2, 0, xt)
                    chA.step(6)
                    v_group(2, 1, xt)
                    chA.step(7)
                    v_group(2, 2, xt)
                    chA.step(8)
                    v_group(2, 3, xt)
                    chA.step(9)
                    chA.step(10)
                    chA.step(11)
                    qk_group(2, 5, xt)
                    chB.step(4)
                    chA.spill()
                    for kt in range(5, 12):
                        chB.step(kt)
                    chB.spill()
                    for o in (6, 7):
                        qk_group(2, o, xt)
                else:
                    qk_group(3, 6, xt)
                    chC.step(8)
                    chC.step(9)
                    v_group(3, 0, xt)
                    chC.step(10)
                    v_group(3, 1, xt)
                    chC.step(11)
                    v_group(3, 2, xt)
                    chC.step(12)
                    v_group(3, 3, xt)
                    chC.step(13)
                    chC.step(14)
                    chC.step(15)
                    qk_group(3, 7, xt)
                    chD.step(8)
                    chC.finish(at0)
                    for kt in range(9, 16):
                        chD.step(kt)
                    qk_group(3, 4, xt)
                    chA.step(12)
                    chD.finish(at0)
                    chA.step(13)
                    chA.step(14)
                    chA.step(15)
                    qk_group(3, 5, xt)
                    chB.step(12)
                    chA.finish(at0)
                    chB.step(13)
                    chB.step(14)
                    chB.step(15)
                    qk_group(1, 0, xts[1])
                    chB.finish(at0)

            # w_proj is first read by the projection, deep into the
            # attention phase; loading it here keeps the head-of-queue DMA
            # slots for the x tiles the qkv matmuls are waiting on.
            wp_sb = wres_pool.tile([128, PAIRS, D], F16)
            nc.sync.dma_start(wp_sb[:, :, :], wproj[:, :, :])

            def proj(at_t, qn_t):
                for s in range(4):
                    t0 = qn_t * 512 + s * 128
                    for e in range(2):
                        op_ = p5_pool.tile([128, 512], F32, tag="p512", name="op")
                        for p_ in range(PAIRS):
                            nc.tensor.matmul(
                                op_[:, :],
                                lhsT=at_t[:, p_, s * 128:(s + 1) * 128],
                                rhs=wp_sb[:, p_, e * 512:(e + 1) * 512],
                                start=(p_ == 0),
                                stop=(p_ == PAIRS - 1),
                            )
                        ob = ob_pool.tile([128, 512], F32, tag="ob")
                        nc.vector.tensor_copy(ob[:, :], op_[:, :])
                        nc.sync.dma_start(
                            out[t0:t0 + 128, e * 512:(e + 1) * 512], ob[:, :])

            # ---- attention + projection, one 512-query tile at a time.
            # The stretch is ACT(exp)-bound; JIT q projections for qn+1,
            # proj(qn-1) matmul groups, and output copies are queued as side
            # work and pumped one item per odd k-tile into the PE-idle slack.
            side = []

            def pump():
                if side:
                    side.pop(0)()

            def proj_side(at_t, qn_t):
                work = []
                for s in range(4):
                    t0 = qn_t * 512 + s * 128
                    for e in range(2):
                        work.append(
                            lambda s=s, e=e, t0=t0: proj_group(at_t, t0, s, e))
                return work

            def proj_group(at_t, t0, s, e):
                op_ = p5_pool.tile([128, 512], F32, tag="p512", name="op")
                for p_ in range(PAIRS):
                    nc.tensor.matmul(
                        op_[:, :],
                        lhsT=at_t[:, p_, s * 128:(s + 1) * 128],
                        rhs=wp_sb[:, p_, e * 512:(e + 1) * 512],
                        start=(p_ == 0),
                        stop=(p_ == PAIRS - 1),
                    )
                ob = ob_pool.tile([128, 512], F32, tag="ob")
                nc.vector.tensor_copy(ob[:, :], op_[:, :])
                nc.sync.dma_start(
                    out[t0:t0 + 128, e * 512:(e + 1) * 512], ob[:, :])

            # Flat phase-2 plan over qn1..3; each chain's finish is
            # deferred until two steps into the next chain so the exp stream
            # never breaks at a chain boundary. JIT q projections and proj
            # output groups ride the side-work queue, pumped one item per
            # odd k-tile into the ACT-bound stretch's PE-idle slack.
            plan = []
            ats = {0: at0}
            for qn in range(1, NQ):
                ats[qn] = at_pool.tile([128, PAIRS, 512], F16, tag="at",
                                       name="at")
                plan += [(Chain(p, qn), range(NK), qn) for p in range(PAIRS)]

            for o in (1, 2, 3):
                side.extend(qk_side(1, o))
            side.extend(proj_side(ats[0], 0))
            for o in range(4):
                side.extend(qk_side(2, o))

            deferred = None   # (chain, qn) awaiting finish
            for ch, kts, qn in plan:
                if qn == 2 and ch.p == 0:
                    side.extend(proj_side(ats[1], 1))
                    for o in range(4):
                        side.extend(qk_side(3, o))
                if qn == 3 and ch.p == 0:
                    side.extend(proj_side(ats[2], 2))
                for idx, kt in enumerate(kts):
                    ch.step(kt)
                    if idx == 1 and deferred is not None:
                        dch, dqn = deferred
                        dch.finish(ats[dqn])
                        deferred = None
                    if idx >= 3 and kt % 3 == 1:
                        pump()
                deferred = (ch, qn)
            dch, dqn = deferred
            dch.finish(ats[dqn])
            while side:
                pump()
            proj(ats[NQ - 1], NQ - 1)
    _orig_to_json = nc.to_json_bytes
    nc.to_json_bytes = lambda: _split_multiwait_matmuls(_orig_to_json())
    return nc


def shard_inputs(x, w_qkv, b_qkv, w_proj, N=N_FULL):
    """Build the 8 per-core input maps from full inputs."""
    x = np.ascontiguousarray(np.asarray(x, dtype=np.float32))
    w_qkv = np.asarray(w_qkv, dtype=np.float32)
    b_qkv = np.asarray(b_qkv, dtype=np.float32)
    w_proj = np.asarray(w_proj, dtype=np.float32)
    in_maps = []
    for c in range(NCORES):
        b, g = divmod(c, 2)
        qc = slice(g * DL, (g + 1) * DL)
        wq = w_qkv[:, 0 * D:1 * D][:, qc]
        wk = w_qkv[:, 1 * D:2 * D][:, qc]
        wv_ = w_qkv[:, 2 * D:3 * D][:, qc]
        wqk_np = np.empty((128, 8, 8, 128), np.float32)
        bqk_np = np.empty((128, 8), np.float32)
        for o in range(8):
            src = wq if o < 4 else wk
            bsrc = b_qkv[0:D][qc] if o < 4 else b_qkv[D:2 * D][qc]
            blk = src[:, (o % 4) * 128:(o % 4 + 1) * 128].reshape(8, 128, 128)
            wqk_np[:, o] = blk.transpose(1, 0, 2)
            bqk_np[:, o] = bsrc[(o % 4) * 128:(o % 4 + 1) * 128]
        wv_np = np.ascontiguousarray(wv_.reshape(8, 128, DL).transpose(1, 0, 2))
        bv_np = np.broadcast_to(b_qkv[2 * D:3 * D][qc], (128, DL)).copy()
        wp_np = np.ascontiguousarray(
            w_proj[g * DL:(g + 1) * DL, :].reshape(PAIRS, 128, D).transpose(1, 0, 2)
        )
        xb = x[min(b, x.shape[0] - 1), :N] if x.ndim == 3 else x[:N]
        in_maps.append({
            "x": np.ascontiguousarray(xb).astype(np.float16),
            "wqk": wqk_np.astype(np.float16),
            "wv": wv_np.astype(np.float16),
            "bqk": bqk_np,
            "bv": bv_np,
            "wproj": wp_np.astype(np.float16),
        })
    return in_maps


_NC_CACHE = {}


def kernel(x, w_qkv, b_qkv, w_proj, b_proj):
    global LAST_EXEC_NS
    x = np.asarray(x, dtype=np.float32)
    b_proj = np.asarray(b_proj, dtype=np.float32)
    if N_FULL not in _NC_CACHE:
        _NC_CACHE[N_FULL] = build(N_FULL)
    nc = _NC_CACHE[N_FULL]
    in_maps = shard_inputs(x, w_qkv, b_qkv, w_proj)
    trace = os.environ.get("KERNEL_TRACE", "0") == "1"
    res = run_bass_kernel_spmd(
        nc, in_maps, core_ids=list(range(NCORES)), trace=trace,
        trace_cores=[0] if trace else None,
    )
    LAST_EXEC_NS = res.exec_time_ns
    outs = [r["out"] for r in res.results]
    full = np.empty((B, N_FULL, D), np.float32)
    for b in range(B):
        full[b] = outs[2 * b] + outs[2 * b + 1]
    full += b_proj[None, None, :]
    return full


# revision 5
# speedup vs baseline: 1.2785x; 1.0179x over previous
"""Multi-head attention block (B=4, N=2048, D=1024, H=16) on 8 trn2 NeuronCores.

Sharding: core c -> (batch b = c//2, head-group g = c%2). Each core computes
attention for 8 heads of one batch plus the partial output projection over its
512 head-dims; the host sums the two partials per batch and adds b_proj.

Per-core kernel (fp16 data path, fp32 PSUM accumulation):
  1. x is cast to fp16 on the host; xT tiles arrive via XBAR DMA transpose
     straight from DRAM (no PE transposes, no PSUM->SBUF copies).
  2. qT/kT computed head-transposed ([dims, tokens], lhsT = w slice),
     bias-added into fp16; v computed natural ([tokens, dims]) with a ones
     column per (k-tile, head) group (v_aug) so the flipped PV matmul also
     yields the softmax denominator.
  3. S^T tiles [k=128, q=512] for the two heads of a pair via two
     row-group-packed matmuls into one 2-bank PSUM tile [128, 1024].
  4. E = exp(scale * S^T) on ScalarE straight out of PSUM into fp16.
  5. PV flipped: out[q, d] per (head, 128-query block): lhsT = E slice
     [128k, 128q], rhs = v_aug slice [128k, 65]; 65-column matmuls
     accumulate over k-tiles. PV for k-tile j is emitted after S^T of
     k-tile j+1 so the in-order PE queue never stalls on the exp.
  6. Tail: 2 reciprocals + 2 broadcast multiplies normalize into fp16 u
     tiles; XBAR DMA transpose writes them back as [dims, tokens] for the
     fp16 projection.
"""

import os
import sys

import numpy as np

try:
    import concourse.bass as bass
except ImportError:  # harness runs from a bare directory
    sys.path.insert(0, "/opt/trn_rl_repo")
    import concourse.bass as bass

import concourse.mybir as mybir
import concourse.tile as tile
from concourse.bass_utils import run_bass_kernel_spmd
from concourse.masks import make_identity

F32 = mybir.dt.float32
F16 = mybir.dt.float16
EXP = mybir.ActivationFunctionType.Exp
ADD = mybir.AluOpType.add
MULT = mybir.AluOpType.mult

B, N_FULL, D = 4, 2048, 1024
H, HD = 16, 64
NCORES = 8
GROUPS = 2          # head-groups (tensor parallel)
HL = H // GROUPS    # 8 heads per core
DL = HL * HD        # 512 local head-dims per core
PAIRS = HL // 2     # 4 head pairs
SCALE = HD ** -0.5
VG = HD + 1         # v dims + ones column per (k-tile, head)

LAST_EXEC_NS = None
EXPS = []  # debug: (p, qn, kt) per emitted exp, in ACT-stream order


def _split_multiwait_matmuls(raw: bytes) -> bytes:
    """This container's walrus allows at most one sync-wait per Matmult.

    Tile attaches up to 3. Hoist the extras onto standalone EventSemaphore
    instructions inserted immediately before the matmul on the same engine
    (identical semantics: the sequencer blocks on them in program order).
    """
    import json

    bir = json.loads(raw)
    n = [0]

    def fix_block(block):
        insts = block.get("instructions")
        if not isinstance(insts, list):
            return
        out = []
        for ins in insts:
            si = ins.get("sync_info") if isinstance(ins, dict) else None
            if (
                isinstance(ins, dict)
                and ins.get("opcode") != "EventSemaphore"
                and si
                and len(si.get("on_wait") or []) > 1
            ):
                waits = si["on_wait"]
                for w in waits[1:]:
                    n[0] += 1
                    out.append({
                        "debug": ins.get("debug", 0),
                        "engine": ins["engine"],
                        "ins": [],
                        "name": f"I-waitfix-{n[0]}",
                        "opcode": "EventSemaphore",
                        "outs": [],
                        "sync_info": {"on_update": [], "on_wait": [w]},
                    })
                si["on_wait"] = waits[:1]
            out.append(ins)
        block["instructions"] = out

    for fn in bir.get("functions", []):
        for block in fn.get("blocks", []):
            fix_block(block)
    return json.dumps(bir).encode()


def build(N=N_FULL):
    NK = N // 128   # k tiles of 128
    NQ = N // 512   # q tiles of 512
    NTT = N // 512  # token tiles of 512 for the qkv projection

    nc = bass.Bass("TRN2", target_bir_lowering=False)
    x = nc.dram_tensor("x", [N, D], F16, kind="ExternalInput")
    # [ii, otile(4 q-pairs then 4 k-pairs), io, 128] so each DMA slab is
    # contiguous per partition.
    wqk = nc.dram_tensor("wqk", [128, 8, 8, 128], F16, kind="ExternalInput")
    wv = nc.dram_tensor("wv", [128, 8, DL], F16, kind="ExternalInput")
    bqk = nc.dram_tensor("bqk", [128, 8], F32, kind="ExternalInput")
    bv = nc.dram_tensor("bv", [128, DL], F32, kind="ExternalInput")
    wproj = nc.dram_tensor("wproj", [128, PAIRS, D], F16, kind="ExternalInput")
    out = nc.dram_tensor("out", [N, D], F16, kind="ExternalOutput")
    # partial projection (pairs 0-1) of the last query tile; the host adds
    # it onto out[3*512:], letting most of the final proj leave the tail
    out2 = nc.dram_tensor("out2", [512, D], F16, kind="ExternalOutput")

    with tile.TileContext(nc) as tc:
        with (
            tc.tile_pool(name="const", bufs=1) as const_pool,
            tc.tile_pool(name="wres", bufs=1) as wres_pool,
            tc.tile_pool(name="wqs", bufs=2) as wqs_pool,
            tc.tile_pool(name="xt", bufs=4) as xt_pool,
            tc.tile_pool(name="qk", bufs=1) as qk_pool,
            tc.tile_pool(name="vg", bufs=1) as vg_pool,
            tc.tile_pool(name="at", bufs=2) as at_pool,
            tc.tile_pool(name="ep", bufs=4) as e_pool,
            tc.tile_pool(name="rp", bufs=2) as r_pool,
            tc.tile_pool(name="up", bufs=4) as u_pool,
            tc.tile_pool(name="sg", bufs=10) as sg_pool,
            tc.tile_pool(name="ob", bufs=2) as ob_pool,
            tc.tile_pool(name="psst", bufs=2, space="PSUM") as pss_pool,
            tc.tile_pool(name="pspv", bufs=2, space="PSUM") as psv_pool,
            tc.tile_pool(name="ps512", bufs=2, space="PSUM") as p5_pool,
        ):
            # Warm up the ACT exp table while the first DMAs are in flight so
            # the first real exp doesn't pay the table load.
            warm = const_pool.tile([128, 1], F32)
            nc.gpsimd.memset(warm[:, :], 0.0)
            nc.scalar.activation(warm[:, :], warm[:, :], EXP)

            bqk_sb = const_pool.tile([128, 8], F32)

            # fp16 identity for the PE transposes in the attention tail
            # (fp16 memset is ISA-invalid, so build in f32 and cast)
            ident32 = const_pool.tile([128, 128], F32)
            make_identity(nc, ident32[:, :])
            ident16 = const_pool.tile([128, 128], F16)
            nc.vector.tensor_copy(ident16[:, :], ident32[:, :])

            qT = qk_pool.tile([128, PAIRS, N], F16, tag="qT")
            kT = qk_pool.tile([128, PAIRS, N], F16, tag="kT")
            # Flat v layout: per (k-tile, head) a 65-column group = 64 v-dims
            # + ones column (PV denominator column after the flip).
            vaug = vg_pool.tile([128, NK * HL * VG], F16, tag="vaug")
            ones_view = vaug[:, :].rearrange(
                "p (g c) -> p g c", c=VG)[:, :, HD:HD + 1]
            nc.vector.tensor_scalar(
                out=ones_view, in0=warm[:, None, 0:1].broadcast_to(
                    [128, NK * HL, 1]),
                scalar1=0.0, scalar2=1.0, op0=MULT, op1=ADD,
            )

            class Chain:
                """One (pair, qn) attention chain, PV pipelined one kt back.

                Can be spilled mid-way: the PSUM partials move to SBUF
                segment tiles so another chain can use the PSUM banks, and
                segments are merged back in finish().
                """

                def __init__(self, p, qn):
                    self.p, self.qn = p, qn
                    self.pvA = self.pvB = None
                    self.segs = [None, None]
                    self.prev = None   # (e2, kt) awaiting its PV
                    self.first = True

                def _pvs(self):
                    return ((0, self.pvA), (1, self.pvB))

                def _pv(self):
                    e2, kt = self.prev
                    self.prev = None
                    for hh, pv in self._pvs():
                        vo = (kt * HL + 2 * self.p + hh) * VG
                        for qs in range(4):
                            nc.tensor.matmul(
                                pv[:, qs, :],
                                lhsT=e2[:, hh * 512 + qs * 128:
                                        hh * 512 + (qs + 1) * 128],
                                rhs=vaug[:, vo:vo + VG],
                                start=(self.first and qs == 0),
                                stop=False,
                                skip_group_check=True,
                            )
                        if hh == 1:
                            self.first = False

                def step(self, kt):
                    if self.pvA is None:
                        self.pvA = psv_pool.tile([128, 4, VG], F32, tag="pv",
                                                 name="pvA")
                        self.pvB = psv_pool.tile([128, 4, VG], F32, tag="pv",
                                                 name="pvB")
                        self.first = True
                    q0 = self.qn * 512
                    k0 = kt * 128
                    stab = pss_pool.tile([128, 1024], F32, tag="st",
                                         name="stab")
                    for fo, base in ((0, 0), (512, 64)):
                        nc.tensor.matmul(
                            stab[:, fo:fo + 512],
                            lhsT=kT[base:base + 64, self.p, k0:k0 + 128],
                            rhs=qT[base:base + 64, self.p, q0:q0 + 512],
                            start=True,
                            stop=True,
                            tile_position=(base, 0),
                            skip_group_check=True,
                        )
                    e2 = e_pool.tile([128, 1024], F16, tag="e", name="e2")
                    nc.scalar.activation(e2[:, :], stab[:, :], EXP, scale=SCALE)
                    EXPS.append((self.p, self.qn, kt))
                    if self.prev is not None:
                        self._pv()
                    self.prev = (e2, kt)

                def spill(self):
                    """Drain the pending PV and move partials to SBUF."""
                    if self.prev is not None:
                        self._pv()
                    for hh, pv in self._pvs():
                        if self.segs[hh] is None:
                            seg = sg_pool.tile([128, 4, VG], F32, tag="sg",
                                               name="seg")
                            nc.vector.tensor_copy(seg[:, :, :], pv[:, :, :])
                        else:
                            seg = sg_pool.tile([128, 4, VG], F32, tag="sg",
                                               name="seg")
                            nc.vector.tensor_tensor(
                                out=seg[:, :, :], in0=pv[:, :, :],
                                in1=self.segs[hh][:, :, :], op=ADD)
                        self.segs[hh] = seg
                    self.pvA = self.pvB = None

                def finish(self, at_t):
                    if self.prev is not None:
                        self._pv()
                    p = self.p
                    # merge spilled segments, then normalize by the
                    # per-query denominator (column 64) during the fp16 copy
                    rcs = r_pool.tile([128, 2, 4, 1], F32, tag="rc", name="rcs")
                    srcs = []
                    for hh, pv in self._pvs():
                        if self.segs[hh] is not None:
                            fin = sg_pool.tile([128, 4, VG], F32, tag="sg",
                                               name="fin")
                            nc.vector.tensor_tensor(
                                out=fin[:, :, :], in0=pv[:, :, :],
                                in1=self.segs[hh][:, :, :], op=ADD)
                            srcs.append(fin)
                        else:
                            srcs.append(pv)
                    for hh, src in enumerate(srcs):
                        nc.vector.reciprocal(
                            rcs[:, hh, :, :], src[:, :, HD:HD + 1])
                    tr = p5_pool.tile([128, 512], F32, tag="p512", name="tr")
                    for hh, src in enumerate(srcs):
                        u = u_pool.tile([128, 4, HD], F16, tag="u", name="u")
                        nc.vector.tensor_tensor(
                            out=u[:, :, :],
                            in0=src[:, :, 0:HD],
                            in1=rcs[:, hh, :, :].broadcast_to([128, 4, HD]),
                            op=MULT,
                        )
                        for qs in range(4):
                            nc.tensor.matmul(
                                tr[hh * 64:(hh + 1) * 64,
                                   qs * 128:(qs + 1) * 128],
                                lhsT=u[:, qs, :],
                                rhs=ident16[:, :],
                                start=True,
                                stop=True,
                                skip_group_check=True,
                            )
                    nc.vector.tensor_copy(at_t[:, p, :], tr[:, :])
                    self.pvA = self.pvB = None
                    self.segs = [None, None]

            # Early chain: (pair 0, qn 0) runs during the kv phase — its
            # k-tiles become valid t-tile by t-tile, so its exps fill the
            # otherwise ACT-idle prefix.
            at0 = at_pool.tile([128, PAIRS, 512], F16, tag="at", name="at0")

            xts = []

            def qk_quarter(ti, o, state, q, xt):
                """Two ics of a q/k projection block, for side-work
                pumping: keeps per-pump PE cost at ~0.43us."""
                if q == 0:
                    wo = wqs_pool.tile([128, 8, 128], F16, tag="wo")
                    nc.sync.dma_start(wo[:, :, :], wqk[:, o, :, :])
                    qp = p5_pool.tile([128, 512], F32, tag="p512", name="qp")
                    state[:] = [wo, qp]
                wo, qp = state
                for ic in range(q * 2, q * 2 + 2):
                    nc.tensor.matmul(
                        qp[:, :],
                        lhsT=wo[:, ic, :],
                        rhs=xt[:, ic, :],
                        start=(ic == 0),
                        stop=(ic == 7),
                    )
                if q == 3:
                    dst = qT if o < 4 else kT
                    nc.vector.tensor_scalar_add(
                        dst[:, o % 4, ti * 512:(ti + 1) * 512], qp[:, :],
                        bqk_sb[:, o:o + 1],
                    )

            def qk_side(ti, o):
                state = []
                return [lambda q=q: qk_quarter(ti, o, state, q, xts[ti])
                        for q in (0, 1, 2, 3)]

            def qk_group(ti, o, xt):
                """One 128-dim output block of the q/k projection."""
                wo = wqs_pool.tile([128, 8, 128], F16, tag="wo")
                nc.sync.dma_start(wo[:, :, :], wqk[:, o, :, :])
                qp = p5_pool.tile([128, 512], F32, tag="p512", name="qp")
                for ic in range(8):
                    nc.tensor.matmul(
                        qp[:, :],
                        lhsT=wo[:, ic, :],
                        rhs=xt[:, ic, :],
                        start=(ic == 0),
                        stop=(ic == 7),
                    )
                dst = qT if o < 4 else kT
                nc.vector.tensor_scalar_add(
                    dst[:, o % 4, ti * 512:(ti + 1) * 512], qp[:, :],
                    bqk_sb[:, o:o + 1],
                )

            # ---- phase 1: xT (DMA transpose from DRAM), v, kT for every
            # token tile, plus the qn0 queries; q projections for qn1..3 are
            # deferred into the ACT-bound attention stretch (JIT q). The qn0
            # chains run here against the PE-dense stretch, spilling their
            # PSUM partials so only one chain holds banks at a time.
            chains0 = [Chain(p, 0) for p in range(PAIRS)]
            chA, chB, chC, chD = chains0

            def v_group(ti, s, xt):
                r = ti * 4 + s
                vp = p5_pool.tile([128, DL], F32, tag="p512", name="vp")
                for ic in range(8):
                    nc.tensor.matmul(
                        vp[:, :],
                        lhsT=xt[:, ic, s * 128:(s + 1) * 128],
                        rhs=wv_sb[:, ic, :],
                        start=(ic == 0),
                        stop=(ic == 7),
                    )
                nc.vector.tensor_tensor(
                    out=vaug[:, r * HL * VG:(r + 1) * HL * VG].rearrange(
                        "p (h c) -> p h c", c=VG)[:, :, 0:HD],
                    in0=vp[:, :].rearrange("p (h d) -> p h d", h=HL),
                    in1=bv_sb[:, :].rearrange("p (h d) -> p h d", h=HL),
                    op=ADD,
                )

            for ti in range(NTT):
                xt = xt_pool.tile([128, 8, 512], F16, tag="xt",
                                  name=f"xt{ti}")
                xts.append(xt)
                if ti == 0:
                    # two half-tile transposes so the first q/k matmuls can
                    # start while the second half is still in flight; the
                    # first wo tiles are prefetched between them
                    nc.sync.dma_start_transpose(
                        xt[:, :, 0:256], x[0:256, :])
                    wos0 = {}
                    for o in (4, 0):
                        wo_pre = wqs_pool.tile([128, 8, 128], F16, tag="wo",
                                               name=f"wo_pre{o}")
                        nc.sync.dma_start(wo_pre[:, :, :], wqk[:, o, :, :])
                        wos0[o] = wo_pre
                    nc.sync.dma_start_transpose(
                        xt[:, :, 256:512], x[256:512, :])
                    nc.sync.dma_start(bqk_sb[:, :], bqk[:, :])
                else:
                    nc.sync.dma_start_transpose(
                        xt[:, :, :], x[ti * 512:(ti + 1) * 512, :])
                if ti == 0:
                    # queries/keys for pair 0 first so the exp stream starts
                    # as early as possible; chain B follows A inside ti0.
                    for o in (4, 0):
                        wo = wos0[o]
                        qp = p5_pool.tile([128, 512], F32, tag="p512",
                                          name="qp")
                        for half in (0, 1):
                            for ic in range(8):
                                nc.tensor.matmul(
                                    qp[:, half * 256:(half + 1) * 256],
                                    lhsT=wo[:, ic, :],
                                    rhs=xt[:, ic, half * 256:(half + 1) * 256],
                                    start=(half == 0 and ic == 0),
                                    stop=(half == 1 and ic == 7),
                                    skip_group_check=True,
                                )
                        dst = qT if o < 4 else kT
                        nc.vector.tensor_scalar_add(
                            dst[:, o % 4, 0:512], qp[:, :], bqk_sb[:, o:o + 1])
                    bv_sb = const_pool.tile([128, DL], F32)
                    nc.sync.dma_start(bv_sb[:, :], bv[:, :])
                    wv_sb = wres_pool.tile([128, 8, DL], F16)
                    nc.sync.dma_start(wv_sb[:, :, :], wv[:, :, :])
                    chA.step(0)
                    qk_group(0, 5, xt)
                    qk_group(0, 1, xt)
                    v_group(0, 0, xt)
                    chA.step(1)
                    v_group(0, 1, xt)
                    chA.step(2)
                    v_group(0, 2, xt)
                    chA.step(3)
                    v_group(0, 3, xt)
                    chB.step(0)
                    chA.spill()
                    chB.step(1)
                    chB.step(2)
                    chB.step(3)
                    chB.spill()
                    for o in (6, 2, 7, 3):
                        qk_group(0, o, xt)
                elif ti == 1:
                    qk_group(1, 6, xt)
                    chC.step(0)
                    chC.step(1)
                    v_group(1, 0, xt)
                    chC.step(2)
                    v_group(1, 1, xt)
                    chC.step(3)
                    v_group(1, 2, xt)
                    chC.step(4)
                    v_group(1, 3, xt)
                    chC.step(5)
                    chC.step(6)
                    chC.step(7)
                    qk_group(1, 7, xt)
                    chD.step(0)
                    chC.spill()
                    for kt in range(1, 8):
                        chD.step(kt)
                    chD.spill()
                    for o in (4, 5):
                        qk_group(1, o, xt)
                elif ti == 2:
                    qk_group(2, 4, xt)
                    chA.step(4)
                    chA.step(5)
                    v_group(2, 0, xt)
                    chA.step(6)
                    v_group(2, 1, xt)
                    chA.step(7)
                    v_group(2, 2, xt)
                    chA.step(8)
                    v_group(2, 3, xt)
                    chA.step(9)
                    chA.step(10)
                    chA.step(11)
                    qk_group(2, 5, xt)
                    chB.step(4)
                    chA.spill()
                    for kt in range(5, 12):
                        chB.step(kt)
                    chB.spill()
                    for o in (6, 7):
                        qk_group(2, o, xt)
                else:
                    qk_group(3, 6, xt)
                    chC.step(8)
                    chC.step(9)
                    v_group(3, 0, xt)
                    chC.step(10)
                    v_group(3, 1, xt)
                    chC.step(11)
                    v_group(3, 2, xt)
                    chC.step(12)
                    v_group(3, 3, xt)
                    chC.step(13)
                    chC.step(14)
                    chC.step(15)
                    qk_group(3, 7, xt)
                    chD.step(8)
                    chC.finish(at0)
                    for kt in range(9, 16):
                        chD.step(kt)
                    qk_group(3, 4, xt)
                    chA.step(12)
                    chD.finish(at0)
                    chA.step(13)
                    chA.step(14)
                    chA.step(15)
                    qk_group(3, 5, xt)
                    chB.step(12)
                    chA.finish(at0)
                    chB.step(13)
                    chB.step(14)
                    chB.step(15)
                    qk_group(1, 0, xts[1])
                    chB.finish(at0)

            # w_proj is first read by the projection, deep into the
            # attention phase; loading it here keeps the head-of-queue DMA
            # slots for the x tiles the qkv matmuls are waiting on.
            wp_sb = wres_pool.tile([128, PAIRS, D], F16)
            nc.sync.dma_start(wp_sb[:, :, :], wproj[:, :, :])

            def proj(at_t, qn_t):
                for s in range(4):
                    t0 = qn_t * 512 + s * 128
                    for e in range(2):
                        op_ = p5_pool.tile([128, 512], F32, tag="p512", name="op")
                        for p_ in range(PAIRS):
                            nc.tensor.matmul(
                                op_[:, :],
                                lhsT=at_t[:, p_, s * 128:(s + 1) * 128],
                                rhs=wp_sb[:, p_, e * 512:(e + 1) * 512],
                                start=(p_ == 0),
                                stop=(p_ == PAIRS - 1),
                            )
                        ob = ob_pool.tile([128, 512], F16, tag="ob")
                        nc.vector.tensor_copy(ob[:, :], op_[:, :])
                        nc.sync.dma_start(
                            out[t0:t0 + 128, e * 512:(e + 1) * 512], ob[:, :])

            # ---- attention + projection, one 512-query tile at a time.
            # The stretch is ACT(exp)-bound; JIT q projections for qn+1,
            # proj(qn-1) matmul groups, and output copies are queued as side
            # work and pumped one item per odd k-tile into the PE-idle slack.
            side = []

            def pump():
                if side:
                    side.pop(0)()

            def proj_part(at_t, t0, s, e, state, half, dst=None):
                if half == 0:
                    state[:] = [p5_pool.tile([128, 512], F32, tag="p512",
                                             name="op")]
                op_ = state[0]
                for p_ in (half * 2, half * 2 + 1):
                    nc.tensor.matmul(
                        op_[:, :],
                        lhsT=at_t[:, p_, s * 128:(s + 1) * 128],
                        rhs=wp_sb[:, p_, e * 512:(e + 1) * 512],
                        start=(p_ == 0),
                        stop=(p_ == PAIRS - 1),
                    )
                if half == 1:
                    ob = ob_pool.tile([128, 512], F16, tag="ob")
                    nc.vector.tensor_copy(ob[:, :], op_[:, :])
                    d = out if dst is None else dst
                    nc.sync.dma_start(
                        d[t0:t0 + 128, e * 512:(e + 1) * 512], ob[:, :])

            def proj_side(at_t, qn_t):
                work = []
                for s in range(4):
                    t0 = qn_t * 512 + s * 128
                    for e in range(2):
                        state = []
                        work.extend(
                            lambda s=s, e=e, t0=t0, st=state, h=h:
                            proj_part(at_t, t0, s, e, st, h)
                            for h in (0, 1))
                return work

            def proj_group(at_t, t0, s, e, pairs=range(PAIRS), dst=None):
                op_ = p5_pool.tile([128, 512], F32, tag="p512", name="op")
                pl = list(pairs)
                for p_ in pl:
                    nc.tensor.matmul(
                        op_[:, :],
                        lhsT=at_t[:, p_, s * 128:(s + 1) * 128],
                        rhs=wp_sb[:, p_, e * 512:(e + 1) * 512],
                        start=(p_ == pl[0]),
                        stop=(p_ == pl[-1]),
                    )
                ob = ob_pool.tile([128, 512], F16, tag="ob")
                nc.vector.tensor_copy(ob[:, :], op_[:, :])
                d = out if dst is None else dst
                nc.sync.dma_start(
                    d[t0:t0 + 128, e * 512:(e + 1) * 512], ob[:, :])

            # Flat phase-2 plan over qn1..3; each chain's finish is
            # deferred until two steps into the next chain so the exp stream
            # never breaks at a chain boundary. JIT q projections and proj
            # output groups ride the side-work queue, pumped one item per
            # odd k-tile into the ACT-bound stretch's PE-idle slack.
            plan = []
            ats = {0: at0}
            for qn in range(1, NQ):
                ats[qn] = at_pool.tile([128, PAIRS, 512], F16, tag="at",
                                       name="at")
                plan += [(Chain(p, qn), range(NK), qn) for p in range(PAIRS)]

            for o in (1, 2, 3):
                side.extend(qk_side(1, o))
            side.extend(qk_side(2, 0))
            side.extend(proj_side(ats[0], 0))

            deferred = None
            for ch, kts, qn in plan:
                if qn == 2 and ch.p == 0:
                    for o in (1, 2, 3):
                        side.extend(qk_side(2, o))
                    side.extend(qk_side(3, 0))
                    side.extend(proj_side(ats[1], 1))
                if qn == 3 and ch.p == 0:
                    for o in (1, 2, 3):
                        side.extend(qk_side(3, o))
                    side.extend(proj_side(ats[2], 2))
                if qn == 3 and ch.p == 2:
                    side.extend(
                        lambda s=s, e=e: proj_group(
                            ats[3], s * 128, s, e, pairs=range(2), dst=out2)
                        for s in range(4) for e in range(2))
                    # (each item is a 2-matmul group + copy: ~0.43us PE)
                for idx, kt in enumerate(kts):
                    ch.step(kt)
                    if idx == 1 and deferred is not None:
                        dch, dqn = deferred
                        dch.finish(ats[dqn])
                        deferred = None
                    if idx >= 2:
                        pump()
                deferred = (ch, qn)
            dch, dqn = deferred
            dch.finish(ats[dqn])
            while side:
                pump()
            for s in range(4):
                for e in range(2):
                    op_ = p5_pool.tile([128, 512], F32, tag="p512", name="op")
                    for p_ in (2, 3):
                        nc.tensor.matmul(
                            op_[:, :],
                            lhsT=ats[3][:, p_, s * 128:(s + 1) * 128],
                            rhs=wp_sb[:, p_, e * 512:(e + 1) * 512],
                            start=(p_ == 2),
                            stop=(p_ == 3),
                        )
                    ob = ob_pool.tile([128, 512], F16, tag="obt",
                                      name="obt", bufs=4)
                    if e == 0:
                        nc.vector.tensor_copy(ob[:, :], op_[:, :])
                        eng = nc.sync
                    else:
                        nc.scalar.activation(
                            ob[:, :], op_[:, :],
                            mybir.ActivationFunctionType.Copy)
                        eng = nc.scalar
                    eng.dma_start(
                        out[3 * 512 + s * 128:3 * 512 + s * 128 + 128,
                            e * 512:(e + 1) * 512], ob[:, :])
    _orig_to_json = nc.to_json_bytes
    nc.to_json_bytes = lambda: _split_multiwait_matmuls(_orig_to_json())
    return nc


def shard_inputs(x, w_qkv, b_qkv, w_proj, N=N_FULL):
    """Build the 8 per-core input maps from full inputs."""
    x = np.ascontiguousarray(np.asarray(x, dtype=np.float32))
    w_qkv = np.asarray(w_qkv, dtype=np.float32)
    b_qkv = np.asarray(b_qkv, dtype=np.float32)
    w_proj = np.asarray(w_proj, dtype=np.float32)
    in_maps = []
    for c in range(NCORES):
        b, g = divmod(c, 2)
        qc = slice(g * DL, (g + 1) * DL)
        wq = w_qkv[:, 0 * D:1 * D][:, qc]
        wk = w_qkv[:, 1 * D:2 * D][:, qc]
        wv_ = w_qkv[:, 2 * D:3 * D][:, qc]
        wqk_np = np.empty((128, 8, 8, 128), np.float32)
        bqk_np = np.empty((128, 8), np.float32)
        for o in range(8):
            src = wq if o < 4 else wk
            bsrc = b_qkv[0:D][qc] if o < 4 else b_qkv[D:2 * D][qc]
            blk = src[:, (o % 4) * 128:(o % 4 + 1) * 128].reshape(8, 128, 128)
            wqk_np[:, o] = blk.transpose(1, 0, 2)
            bqk_np[:, o] = bsrc[(o % 4) * 128:(o % 4 + 1) * 128]
        wv_np = np.ascontiguousarray(wv_.reshape(8, 128, DL).transpose(1, 0, 2))
        bv_np = np.broadcast_to(b_qkv[2 * D:3 * D][qc], (128, DL)).copy()
        wp_np = np.ascontiguousarray(
            w_proj[g * DL:(g + 1) * DL, :].reshape(PAIRS, 128, D).transpose(1, 0, 2)
        )
        xb = x[min(b, x.shape[0] - 1), :N] if x.ndim == 3 else x[:N]
        in_maps.append({
            "x": np.ascontiguousarray(xb).astype(np.float16),
            "wqk": wqk_np.astype(np.float16),
            "wv": wv_np.astype(np.float16),
            "bqk": bqk_np,
            "bv": bv_np,
            "wproj": wp_np.astype(np.float16),
        })
    return in_maps


_NC_CACHE = {}


def kernel(x, w_qkv, b_qkv, w_proj, b_proj):
    global LAST_EXEC_NS
    x = np.asarray(x, dtype=np.float32)
    b_proj = np.asarray(b_proj, dtype=np.float32)
    if N_FULL not in _NC_CACHE:
        _NC_CACHE[N_FULL] = build(N_FULL)
    nc = _NC_CACHE[N_FULL]
    in_maps = shard_inputs(x, w_qkv, b_qkv, w_proj)
    trace = os.environ.get("KERNEL_TRACE", "0") == "1"
    res = run_bass_kernel_spmd(
        nc, in_maps, core_ids=list(range(NCORES)), trace=trace,
        trace_cores=[0] if trace else None,
    )
    LAST_EXEC_NS = res.exec_time_ns
    full = np.empty((B, N_FULL, D), np.float32)
    for b in range(B):
        r0, r1 = res.results[2 * b], res.results[2 * b + 1]
        full[b] = r0["out"].astype(np.float32) + r1["out"].astype(np.float32)
        full[b][3 * 512:] += (r0["out2"].astype(np.float32)
                              + r1["out2"].astype(np.float32))
    full += b_proj[None, None, :]
    return full


# revision 6
# speedup vs baseline: 1.2934x; 1.0116x over previous
"""Multi-head attention block (B=4, N=2048, D=1024, H=16) on 8 trn2 NeuronCores.

Sharding: core c -> (batch b = c//2, head-group g = c%2). Each core computes
attention for 8 heads of one batch plus the partial output projection over its
512 head-dims; the host sums the two partials per batch and adds b_proj.

Per-core kernel (fp16 data path, fp32 PSUM accumulation):
  1. x is cast to fp16 on the host; xT tiles arrive via XBAR DMA transpose
     straight from DRAM (no PE transposes, no PSUM->SBUF copies).
  2. qT/kT computed head-transposed ([dims, tokens], lhsT = w slice),
     bias-added into fp16; v computed natural ([tokens, dims]) with a ones
     column per (k-tile, head) group (v_aug) so the flipped PV matmul also
     yields the softmax denominator.
  3. S^T tiles [k=128, q=512] for the two heads of a pair via two
     row-group-packed matmuls into one 2-bank PSUM tile [128, 1024].
  4. E = exp(scale * S^T) on ScalarE straight out of PSUM into fp16.
  5. PV flipped: out[q, d] per (head, 128-query block): lhsT = E slice
     [128k, 128q], rhs = v_aug slice [128k, 65]; 65-column matmuls
     accumulate over k-tiles. PV for k-tile j is emitted after S^T of
     k-tile j+1 so the in-order PE queue never stalls on the exp.
  6. Tail: 2 reciprocals + 2 broadcast multiplies normalize into fp16 u
     tiles; XBAR DMA transpose writes them back as [dims, tokens] for the
     fp16 projection.
"""

import os
import sys

import numpy as np

try:
    import concourse.bass as bass
except ImportError:  # harness runs from a bare directory
    sys.path.insert(0, "/opt/trn_rl_repo")
    import concourse.bass as bass

import concourse.mybir as mybir
import concourse.tile as tile
from concourse.bass_utils import run_bass_kernel_spmd
from concourse.masks import make_identity

F32 = mybir.dt.float32
F16 = mybir.dt.float16
EXP = mybir.ActivationFunctionType.Exp
ADD = mybir.AluOpType.add
MULT = mybir.AluOpType.mult

B, N_FULL, D = 4, 2048, 1024
H, HD = 16, 64
NCORES = 8
GROUPS = 2          # head-groups (tensor parallel)
HL = H // GROUPS    # 8 heads per core
DL = HL * HD        # 512 local head-dims per core
PAIRS = HL // 2     # 4 head pairs
SCALE = HD ** -0.5
VG = HD + 1         # v dims + ones column per (k-tile, head)

LAST_EXEC_NS = None
EXPS = []  # debug: (p, qn, kt) per emitted exp, in ACT-stream order


def _split_multiwait_matmuls(raw: bytes) -> bytes:
    """This container's walrus allows at most one sync-wait per Matmult.

    Tile attaches up to 3. Hoist the extras onto standalone EventSemaphore
    instructions inserted immediately before the matmul on the same engine
    (identical semantics: the sequencer blocks on them in program order).
    """
    import json

    bir = json.loads(raw)
    n = [0]

    def fix_block(block):
        insts = block.get("instructions")
        if not isinstance(insts, list):
            return
        out = []
        for ins in insts:
            si = ins.get("sync_info") if isinstance(ins, dict) else None
            if (
                isinstance(ins, dict)
                and ins.get("opcode") != "EventSemaphore"
                and si
                and len(si.get("on_wait") or []) > 1
            ):
                waits = si["on_wait"]
                for w in waits[1:]:
                    n[0] += 1
                    out.append({
                        "debug": ins.get("debug", 0),
                        "engine": ins["engine"],
                        "ins": [],
                        "name": f"I-waitfix-{n[0]}",
                        "opcode": "EventSemaphore",
                        "outs": [],
                        "sync_info": {"on_update": [], "on_wait": [w]},
                    })
                si["on_wait"] = waits[:1]
            out.append(ins)
        block["instructions"] = out

    for fn in bir.get("functions", []):
        for block in fn.get("blocks", []):
            fix_block(block)
    return json.dumps(bir).encode()


def build(N=N_FULL):
    NK = N // 128   # k tiles of 128
    NQ = N // 512   # q tiles of 512
    NTT = N // 512  # token tiles of 512 for the qkv projection

    nc = bass.Bass("TRN2", target_bir_lowering=False)
    x = nc.dram_tensor("x", [N, D], F16, kind="ExternalInput")
    # [ii, otile(4 q-pairs then 4 k-pairs), io, 128] so each DMA slab is
    # contiguous per partition.
    wqk = nc.dram_tensor("wqk", [128, 8, 8, 128], F16, kind="ExternalInput")
    wv = nc.dram_tensor("wv", [128, 8, DL], F16, kind="ExternalInput")
    bqk = nc.dram_tensor("bqk", [128, 8], F32, kind="ExternalInput")
    bv = nc.dram_tensor("bv", [128, DL], F32, kind="ExternalInput")
    wproj = nc.dram_tensor("wproj", [128, PAIRS, D], F16, kind="ExternalInput")
    out = nc.dram_tensor("out", [N, D], F16, kind="ExternalOutput")
    # partial projection (pairs 0-1) of the last query tile; the host adds
    # it onto out[3*512:], letting most of the final proj leave the tail
    out2 = nc.dram_tensor("out2", [512, D], F16, kind="ExternalOutput")

    with tile.TileContext(nc) as tc:
        with (
            tc.tile_pool(name="const", bufs=1) as const_pool,
            tc.tile_pool(name="wres", bufs=1) as wres_pool,
            tc.tile_pool(name="wqs", bufs=3) as wqs_pool,
            tc.tile_pool(name="xt", bufs=4) as xt_pool,
            tc.tile_pool(name="qk", bufs=1) as qk_pool,
            tc.tile_pool(name="vg", bufs=1) as vg_pool,
            tc.tile_pool(name="at", bufs=2) as at_pool,
            tc.tile_pool(name="ep", bufs=4) as e_pool,
            tc.tile_pool(name="rp", bufs=2) as r_pool,
            tc.tile_pool(name="up", bufs=6) as u_pool,
            tc.tile_pool(name="sg", bufs=10) as sg_pool,
            tc.tile_pool(name="ob", bufs=3) as ob_pool,
            tc.tile_pool(name="psst", bufs=2, space="PSUM") as pss_pool,
            tc.tile_pool(name="pspv", bufs=2, space="PSUM") as psv_pool,
            tc.tile_pool(name="ps512", bufs=2, space="PSUM") as p5_pool,
        ):
            # Warm up the ACT exp table while the first DMAs are in flight so
            # the first real exp doesn't pay the table load.
            warm = const_pool.tile([128, 1], F32)
            nc.gpsimd.memset(warm[:, :], 0.0)
            nc.scalar.activation(warm[:, :], warm[:, :], EXP)

            bqk_sb = const_pool.tile([128, 8], F32)

            # fp16 identity for the PE transposes in the attention tail
            # (fp16 memset is ISA-invalid, so build in f32 and cast)
            ident32 = const_pool.tile([128, 128], F32)
            make_identity(nc, ident32[:, :])
            ident16 = const_pool.tile([128, 128], F16)
            nc.vector.tensor_copy(ident16[:, :], ident32[:, :])

            qT = qk_pool.tile([128, PAIRS, N], F16, tag="qT")
            kT = qk_pool.tile([128, PAIRS, N], F16, tag="kT")
            # Flat v layout: per (k-tile, head) a 65-column group = 64 v-dims
            # + ones column (PV denominator column after the flip).
            vaug = vg_pool.tile([128, NK * HL * VG], F16, tag="vaug")
            ones_view = vaug[:, :].rearrange(
                "p (g c) -> p g c", c=VG)[:, :, HD:HD + 1]
            nc.vector.tensor_scalar(
                out=ones_view, in0=warm[:, None, 0:1].broadcast_to(
                    [128, NK * HL, 1]),
                scalar1=0.0, scalar2=1.0, op0=MULT, op1=ADD,
            )

            class Chain:
                """One (pair, qn) attention chain, PV pipelined one kt back.

                Can be spilled mid-way: the PSUM partials move to SBUF
                segment tiles so another chain can use the PSUM banks, and
                segments are merged back in finish().
                """

                def __init__(self, p, qn):
                    self.p, self.qn = p, qn
                    self.pvA = self.pvB = None
                    self.segs = [None, None]
                    self.prev = None   # (e2, kt) awaiting its PV
                    self.first = True

                def _pvs(self):
                    return ((0, self.pvA), (1, self.pvB))

                def _pv(self):
                    e2, kt = self.prev
                    self.prev = None
                    for hh, pv in self._pvs():
                        vo = (kt * HL + 2 * self.p + hh) * VG
                        for qs in range(4):
                            nc.tensor.matmul(
                                pv[:, qs, :],
                                lhsT=e2[:, hh * 512 + qs * 128:
                                        hh * 512 + (qs + 1) * 128],
                                rhs=vaug[:, vo:vo + VG],
                                start=(self.first and qs == 0),
                                stop=False,
                                skip_group_check=True,
                            )
                        if hh == 1:
                            self.first = False

                def step(self, kt):
                    if self.pvA is None:
                        self.pvA = psv_pool.tile([128, 4, VG], F32, tag="pv",
                                                 name="pvA")
                        self.pvB = psv_pool.tile([128, 4, VG], F32, tag="pv",
                                                 name="pvB")
                        self.first = True
                    q0 = self.qn * 512
                    k0 = kt * 128
                    stab = pss_pool.tile([128, 1024], F32, tag="st",
                                         name="stab")
                    for fo, base in ((0, 0), (512, 64)):
                        nc.tensor.matmul(
                            stab[:, fo:fo + 512],
                            lhsT=kT[base:base + 64, self.p, k0:k0 + 128],
                            rhs=qT[base:base + 64, self.p, q0:q0 + 512],
                            start=True,
                            stop=True,
                            tile_position=(base, 0),
                            skip_group_check=True,
                        )
                    e2 = e_pool.tile([128, 1024], F16, tag="e", name="e2")
                    nc.scalar.activation(e2[:, :], stab[:, :], EXP, scale=SCALE)
                    EXPS.append((self.p, self.qn, kt))
                    if self.prev is not None:
                        self._pv()
                    self.prev = (e2, kt)

                def spill(self):
                    """Drain the pending PV and move partials to SBUF."""
                    if self.prev is not None:
                        self._pv()
                    for hh, pv in self._pvs():
                        if self.segs[hh] is None:
                            seg = sg_pool.tile([128, 4, VG], F32, tag="sg",
                                               name="seg")
                            nc.vector.tensor_copy(seg[:, :, :], pv[:, :, :])
                        else:
                            seg = sg_pool.tile([128, 4, VG], F32, tag="sg",
                                               name="seg")
                            nc.vector.tensor_tensor(
                                out=seg[:, :, :], in0=pv[:, :, :],
                                in1=self.segs[hh][:, :, :], op=ADD)
                        self.segs[hh] = seg
                    self.pvA = self.pvB = None

                def finish(self, at_t):
                    if self.prev is not None:
                        self._pv()
                    p = self.p
                    # merge spilled segments, then normalize by the
                    # per-query denominator (column 64) during the fp16 copy
                    rcs = r_pool.tile([128, 2, 4, 1], F32, tag="rc", name="rcs")
                    srcs = []
                    for hh, pv in self._pvs():
                        if self.segs[hh] is not None:
                            fin = sg_pool.tile([128, 4, VG], F32, tag="sg",
                                               name="fin")
                            nc.vector.tensor_tensor(
                                out=fin[:, :, :], in0=pv[:, :, :],
                                in1=self.segs[hh][:, :, :], op=ADD)
                            srcs.append(fin)
                        else:
                            srcs.append(pv)
                    for hh, src in enumerate(srcs):
                        nc.vector.reciprocal(
                            rcs[:, hh, :, :], src[:, :, HD:HD + 1])
                    tr = p5_pool.tile([128, 512], F32, tag="p512", name="tr")
                    for hh, src in enumerate(srcs):
                        u = u_pool.tile([128, 4, HD], F16, tag="u", name="u")
                        nc.vector.tensor_tensor(
                            out=u[:, :, :],
                            in0=src[:, :, 0:HD],
                            in1=rcs[:, hh, :, :].broadcast_to([128, 4, HD]),
                            op=MULT,
                        )
                        for qs in range(4):
                            nc.tensor.matmul(
                                tr[hh * 64:(hh + 1) * 64,
                                   qs * 128:(qs + 1) * 128],
                                lhsT=u[:, qs, :],
                                rhs=ident16[:, :],
                                start=True,
                                stop=True,
                                skip_group_check=True,
                            )
                    nc.vector.tensor_copy(at_t[:, p, :], tr[:, :])
                    self.pvA = self.pvB = None
                    self.segs = [None, None]

            # Early chain: (pair 0, qn 0) runs during the kv phase — its
            # k-tiles become valid t-tile by t-tile, so its exps fill the
            # otherwise ACT-idle prefix.
            at0 = at_pool.tile([128, PAIRS, 512], F16, tag="at", name="at0")

            xts = []

            def qk_quarter(ti, o, state, q, xt):
                """Two ics of a q/k projection block, for side-work
                pumping: keeps per-pump PE cost at ~0.43us."""
                if q == 0:
                    wo = wqs_pool.tile([128, 8, 128], F16, tag="wo")
                    nc.sync.dma_start(wo[:, :, :], wqk[:, o, :, :])
                    qp = p5_pool.tile([128, 512], F32, tag="p512", name="qp")
                    state[:] = [wo, qp]
                wo, qp = state
                for ic in range(q * 2, q * 2 + 2):
                    nc.tensor.matmul(
                        qp[:, :],
                        lhsT=wo[:, ic, :],
                        rhs=xt[:, ic, :],
                        start=(ic == 0),
                        stop=(ic == 7),
                    )
                if q == 3:
                    dst = qT if o < 4 else kT
                    nc.vector.tensor_scalar_add(
                        dst[:, o % 4, ti * 512:(ti + 1) * 512], qp[:, :],
                        bqk_sb[:, o:o + 1],
                    )

            def qk_side(ti, o):
                state = []
                return [lambda q=q: qk_quarter(ti, o, state, q, xts[ti])
                        for q in (0, 1, 2, 3)]

            def qk_group(ti, o, xt):
                """One 128-dim output block of the q/k projection."""
                wo = wqs_pool.tile([128, 8, 128], F16, tag="wo")
                nc.sync.dma_start(wo[:, :, :], wqk[:, o, :, :])
                qp = p5_pool.tile([128, 512], F32, tag="p512", name="qp")
                for ic in range(8):
                    nc.tensor.matmul(
                        qp[:, :],
                        lhsT=wo[:, ic, :],
                        rhs=xt[:, ic, :],
                        start=(ic == 0),
                        stop=(ic == 7),
                    )
                dst = qT if o < 4 else kT
                nc.vector.tensor_scalar_add(
                    dst[:, o % 4, ti * 512:(ti + 1) * 512], qp[:, :],
                    bqk_sb[:, o:o + 1],
                )

            # ---- phase 1: xT (DMA transpose from DRAM), v, kT for every
            # token tile, plus the qn0 queries; q projections for qn1..3 are
            # deferred into the ACT-bound attention stretch (JIT q). The qn0
            # chains run here against the PE-dense stretch, spilling their
            # PSUM partials so only one chain holds banks at a time.
            chains0 = [Chain(p, 0) for p in range(PAIRS)]
            chA, chB, chC, chD = chains0

            def v_group(ti, s, xt):
                r = ti * 4 + s
                vp = p5_pool.tile([128, DL], F32, tag="p512", name="vp")
                for ic in range(8):
                    nc.tensor.matmul(
                        vp[:, :],
                        lhsT=xt[:, ic, s * 128:(s + 1) * 128],
                        rhs=wv_sb[:, ic, :],
                        start=(ic == 0),
                        stop=(ic == 7),
                    )
                nc.vector.tensor_tensor(
                    out=vaug[:, r * HL * VG:(r + 1) * HL * VG].rearrange(
                        "p (h c) -> p h c", c=VG)[:, :, 0:HD],
                    in0=vp[:, :].rearrange("p (h d) -> p h d", h=HL),
                    in1=bv_sb[:, :].rearrange("p (h d) -> p h d", h=HL),
                    op=ADD,
                )

            for ti in range(NTT):
                xt = xt_pool.tile([128, 8, 512], F16, tag="xt",
                                  name=f"xt{ti}")
                xts.append(xt)
                if ti == 0:
                    # two half-tile transposes so the first q/k matmuls can
                    # start while the second half is still in flight; the
                    # first wo tiles are prefetched between them
                    nc.sync.dma_start_transpose(
                        xt[:, :, 0:256], x[0:256, :])
                    wos0 = {}
                    for o in (4, 0):
                        wo_pre = wqs_pool.tile([128, 8, 128], F16, tag="wo",
                                               name=f"wo_pre{o}")
                        nc.sync.dma_start(wo_pre[:, :, :], wqk[:, o, :, :])
                        wos0[o] = wo_pre
                    nc.sync.dma_start_transpose(
                        xt[:, :, 256:512], x[256:512, :])
                    nc.sync.dma_start(bqk_sb[:, :], bqk[:, :])
                else:
                    nc.sync.dma_start_transpose(
                        xt[:, :, :], x[ti * 512:(ti + 1) * 512, :])
                if ti == 0:
                    # queries/keys for pair 0 first so the exp stream starts
                    # as early as possible; chain B follows A inside ti0.
                    for o in (4, 0):
                        wo = wos0[o]
                        qp = p5_pool.tile([128, 512], F32, tag="p512",
                                          name="qp")
                        for half in (0, 1):
                            for ic in range(8):
                                nc.tensor.matmul(
                                    qp[:, half * 256:(half + 1) * 256],
                                    lhsT=wo[:, ic, :],
                                    rhs=xt[:, ic, half * 256:(half + 1) * 256],
                                    start=(half == 0 and ic == 0),
                                    stop=(half == 1 and ic == 7),
                                    skip_group_check=True,
                                )
                        dst = qT if o < 4 else kT
                        nc.vector.tensor_scalar_add(
                            dst[:, o % 4, 0:512], qp[:, :], bqk_sb[:, o:o + 1])
                    bv_sb = const_pool.tile([128, DL], F32)
                    nc.sync.dma_start(bv_sb[:, :], bv[:, :])
                    wv_sb = wres_pool.tile([128, 8, DL], F16)
                    nc.sync.dma_start(wv_sb[:, :, :], wv[:, :, :])
                    chA.step(0)
                    qk_group(0, 5, xt)
                    qk_group(0, 1, xt)
                    v_group(0, 0, xt)
                    chA.step(1)
                    v_group(0, 1, xt)
                    chA.step(2)
                    v_group(0, 2, xt)
                    chA.step(3)
                    v_group(0, 3, xt)
                    chB.step(0)
                    chA.spill()
                    chB.step(1)
                    chB.step(2)
                    chB.step(3)
                    chB.spill()
                    for o in (6, 2, 7, 3):
                        qk_group(0, o, xt)
                elif ti == 1:
                    qk_group(1, 6, xt)
                    chC.step(0)
                    chC.step(1)
                    v_group(1, 0, xt)
                    chC.step(2)
                    v_group(1, 1, xt)
                    chC.step(3)
                    v_group(1, 2, xt)
                    chC.step(4)
                    v_group(1, 3, xt)
                    chC.step(5)
                    chC.step(6)
                    chC.step(7)
                    qk_group(1, 7, xt)
                    chD.step(0)
                    chC.spill()
                    for kt in range(1, 8):
                        chD.step(kt)
                    chD.spill()
                    for o in (4, 5):
                        qk_group(1, o, xt)
                elif ti == 2:
                    qk_group(2, 4, xt)
                    chA.step(4)
                    chA.step(5)
                    v_group(2, 0, xt)
                    chA.step(6)
                    v_group(2, 1, xt)
                    chA.step(7)
                    v_group(2, 2, xt)
                    chA.step(8)
                    v_group(2, 3, xt)
                    chA.step(9)
                    chA.step(10)
                    chA.step(11)
                    qk_group(2, 5, xt)
                    chB.step(4)
                    chA.spill()
                    for kt in range(5, 12):
                        chB.step(kt)
                    chB.spill()
                    for o in (6, 7):
                        qk_group(2, o, xt)
                else:
                    qk_group(3, 6, xt)
                    chC.step(8)
                    chC.step(9)
                    v_group(3, 0, xt)
                    chC.step(10)
                    v_group(3, 1, xt)
                    chC.step(11)
                    v_group(3, 2, xt)
                    chC.step(12)
                    v_group(3, 3, xt)
                    chC.step(13)
                    chC.step(14)
                    chC.step(15)
                    qk_group(3, 7, xt)
                    chD.step(8)
                    chC.finish(at0)
                    for kt in range(9, 16):
                        chD.step(kt)
                    qk_group(3, 4, xt)
                    chA.step(12)
                    chD.finish(at0)
                    chA.step(13)
                    chA.step(14)
                    chA.step(15)
                    qk_group(3, 5, xt)
                    chB.step(12)
                    chA.finish(at0)
                    chB.step(13)
                    chB.step(14)
                    chB.step(15)
                    qk_group(1, 0, xts[1])
                    chB.finish(at0)

            # w_proj is first read by the projection, deep into the
            # attention phase; loading it here keeps the head-of-queue DMA
            # slots for the x tiles the qkv matmuls are waiting on.
            wp_sb = wres_pool.tile([128, PAIRS, D], F16)
            nc.sync.dma_start(wp_sb[:, :, :], wproj[:, :, :])

            def proj(at_t, qn_t):
                for s in range(4):
                    t0 = qn_t * 512 + s * 128
                    for e in range(2):
                        op_ = p5_pool.tile([128, 512], F32, tag="p512", name="op")
                        for p_ in range(PAIRS):
                            nc.tensor.matmul(
                                op_[:, :],
                                lhsT=at_t[:, p_, s * 128:(s + 1) * 128],
                                rhs=wp_sb[:, p_, e * 512:(e + 1) * 512],
                                start=(p_ == 0),
                                stop=(p_ == PAIRS - 1),
                            )
                        ob = ob_pool.tile([128, 512], F16, tag="ob")
                        nc.vector.tensor_copy(ob[:, :], op_[:, :])
                        nc.sync.dma_start(
                            out[t0:t0 + 128, e * 512:(e + 1) * 512], ob[:, :])

            # ---- attention + projection, one 512-query tile at a time.
            # The stretch is ACT(exp)-bound; JIT q projections for qn+1,
            # proj(qn-1) matmul groups, and output copies are queued as side
            # work and pumped one item per odd k-tile into the PE-idle slack.
            side = []

            def pump():
                if side:
                    side.pop(0)()

            def proj_part(at_t, t0, s, e, state, half, dst=None):
                if half == 0:
                    state[:] = [p5_pool.tile([128, 512], F32, tag="p512",
                                             name="op")]
                op_ = state[0]
                for p_ in (half * 2, half * 2 + 1):
                    nc.tensor.matmul(
                        op_[:, :],
                        lhsT=at_t[:, p_, s * 128:(s + 1) * 128],
                        rhs=wp_sb[:, p_, e * 512:(e + 1) * 512],
                        start=(p_ == 0),
                        stop=(p_ == PAIRS - 1),
                    )
                if half == 1:
                    ob = ob_pool.tile([128, 512], F16, tag="ob")
                    nc.vector.tensor_copy(ob[:, :], op_[:, :])
                    d = out if dst is None else dst
                    nc.sync.dma_start(
                        d[t0:t0 + 128, e * 512:(e + 1) * 512], ob[:, :])

            def proj_side(at_t, qn_t):
                work = []
                for s in range(4):
                    t0 = qn_t * 512 + s * 128
                    for e in range(2):
                        state = []
                        work.extend(
                            lambda s=s, e=e, t0=t0, st=state, h=h:
                            proj_part(at_t, t0, s, e, st, h)
                            for h in (0, 1))
                return work

            def proj_group(at_t, t0, s, e, pairs=range(PAIRS), dst=None):
                op_ = p5_pool.tile([128, 512], F32, tag="p512", name="op")
                pl = list(pairs)
                for p_ in pl:
                    nc.tensor.matmul(
                        op_[:, :],
                        lhsT=at_t[:, p_, s * 128:(s + 1) * 128],
                        rhs=wp_sb[:, p_, e * 512:(e + 1) * 512],
                        start=(p_ == pl[0]),
                        stop=(p_ == pl[-1]),
                    )
                ob = ob_pool.tile([128, 512], F16, tag="ob")
                nc.vector.tensor_copy(ob[:, :], op_[:, :])
                d = out if dst is None else dst
                nc.sync.dma_start(
                    d[t0:t0 + 128, e * 512:(e + 1) * 512], ob[:, :])

            # Flat phase-2 plan over qn1..3; each chain's finish is
            # deferred until two steps into the next chain so the exp stream
            # never breaks at a chain boundary. JIT q projections and proj
            # output groups ride the side-work queue, pumped one item per
            # odd k-tile into the ACT-bound stretch's PE-idle slack.
            plan = []
            ats = {0: at0}
            for qn in range(1, NQ):
                ats[qn] = at_pool.tile([128, PAIRS, 512], F16, tag="at",
                                       name="at")
                plan += [(Chain(p, qn), range(NK), qn) for p in range(PAIRS)]

            for o in (1, 2, 3):
                side.extend(qk_side(1, o))
            side.extend(qk_side(2, 0))
            side.extend(proj_side(ats[0], 0))

            deferred = None
            for ch, kts, qn in plan:
                if qn == 2 and ch.p == 0:
                    for o in (1, 2, 3):
                        side.extend(qk_side(2, o))
                    side.extend(qk_side(3, 0))
                    side.extend(proj_side(ats[1], 1))
                if qn == 3 and ch.p == 0:
                    for o in (1, 2, 3):
                        side.extend(qk_side(3, o))
                    side.extend(proj_side(ats[2], 2))
                if qn == 3 and ch.p == 2:
                    side.extend(
                        lambda s=s, e=e: proj_group(
                            ats[3], s * 128, s, e, pairs=range(2), dst=out2)
                        for s in range(4) for e in range(2))
                    # (each item is a 2-matmul group + copy: ~0.43us PE)
                for idx, kt in enumerate(kts):
                    ch.step(kt)
                    if idx == 1 and deferred is not None:
                        dch, dqn = deferred
                        dch.finish(ats[dqn])
                        deferred = None
                    if idx >= 5:
                        pump()
                deferred = (ch, qn)
            dch, dqn = deferred
            dch.finish(ats[dqn])
            while side:
                pump()
            for s in range(4):
                for e in range(2):
                    op_ = p5_pool.tile([128, 512], F32, tag="p512", name="op")
                    for p_ in (2, 3):
                        nc.tensor.matmul(
                            op_[:, :],
                            lhsT=ats[3][:, p_, s * 128:(s + 1) * 128],
                            rhs=wp_sb[:, p_, e * 512:(e + 1) * 512],
                            start=(p_ == 2),
                            stop=(p_ == 3),
                        )
                    ob = ob_pool.tile([128, 512], F16, tag="obt",
                                      name="obt", bufs=4)
                    if e == 0:
                        nc.vector.tensor_copy(ob[:, :], op_[:, :])
                        eng = nc.sync
                    else:
                        nc.scalar.activation(
                            ob[:, :], op_[:, :],
                            mybir.ActivationFunctionType.Copy)
                        eng = nc.scalar
                    eng.dma_start(
                        out[3 * 512 + s * 128:3 * 512 + s * 128 + 128,
                            e * 512:(e + 1) * 512], ob[:, :])
    _orig_to_json = nc.to_json_bytes
    nc.to_json_bytes = lambda: _split_multiwait_matmuls(_orig_to_json())
    return nc


def shard_inputs(x, w_qkv, b_qkv, w_proj, N=N_FULL):
    """Build the 8 per-core input maps from full inputs."""
    x = np.ascontiguousarray(np.asarray(x, dtype=np.float32))
    w_qkv = np.asarray(w_qkv, dtype=np.float32)
    b_qkv = np.asarray(b_qkv, dtype=np.float32)
    w_proj = np.asarray(w_proj, dtype=np.float32)
    in_maps = []
    for c in range(NCORES):
        b, g = divmod(c, 2)
        qc = slice(g * DL, (g + 1) * DL)
        wq = w_qkv[:, 0 * D:1 * D][:, qc]
        wk = w_qkv[:, 1 * D:2 * D][:, qc]
        wv_ = w_qkv[:, 2 * D:3 * D][:, qc]
        wqk_np = np.empty((128, 8, 8, 128), np.float32)
        bqk_np = np.empty((128, 8), np.float32)
        for o in range(8):
            src = wq if o < 4 else wk
            bsrc = b_qkv[0:D][qc] if o < 4 else b_qkv[D:2 * D][qc]
            blk = src[:, (o % 4) * 128:(o % 4 + 1) * 128].reshape(8, 128, 128)
            wqk_np[:, o] = blk.transpose(1, 0, 2)
            bqk_np[:, o] = bsrc[(o % 4) * 128:(o % 4 + 1) * 128]
        wv_np = np.ascontiguousarray(wv_.reshape(8, 128, DL).transpose(1, 0, 2))
        bv_np = np.broadcast_to(b_qkv[2 * D:3 * D][qc], (128, DL)).copy()
        wp_np = np.ascontiguousarray(
            w_proj[g * DL:(g + 1) * DL, :].reshape(PAIRS, 128, D).transpose(1, 0, 2)
        )
        xb = x[min(b, x.shape[0] - 1), :N] if x.ndim == 3 else x[:N]
        in_maps.append({
            "x": np.ascontiguousarray(xb).astype(np.float16),
            "wqk": wqk_np.astype(np.float16),
            "wv": wv_np.astype(np.float16),
            "bqk": bqk_np,
            "bv": bv_np,
            "wproj": wp_np.astype(np.float16),
        })
    return in_maps


_NC_CACHE = {}


def kernel(x, w_qkv, b_qkv, w_proj, b_proj):
    global LAST_EXEC_NS
    x = np.asarray(x, dtype=np.float32)
    b_proj = np.asarray(b_proj, dtype=np.float32)
    if N_FULL not in _NC_CACHE:
        _NC_CACHE[N_FULL] = build(N_FULL)
    nc = _NC_CACHE[N_FULL]
    in_maps = shard_inputs(x, w_qkv, b_qkv, w_proj)
    trace = os.environ.get("KERNEL_TRACE", "0") == "1"
    res = run_bass_kernel_spmd(
        nc, in_maps, core_ids=list(range(NCORES)), trace=trace,
        trace_cores=[0] if trace else None,
    )
    LAST_EXEC_NS = res.exec_time_ns
    full = np.empty((B, N_FULL, D), np.float32)
    for b in range(B):
        r0, r1 = res.results[2 * b], res.results[2 * b + 1]
        full[b] = r0["out"].astype(np.float32) + r1["out"].astype(np.float32)
        full[b][3 * 512:] += (r0["out2"].astype(np.float32)
                              + r1["out2"].astype(np.float32))
    full += b_proj[None, None, :]
    return full


# revision 7
# speedup vs baseline: 1.2952x; 1.0014x over previous
"""Multi-head attention block (B=4, N=2048, D=1024, H=16) on 8 trn2 NeuronCores.

Sharding: core c -> (batch b = c//2, head-group g = c%2). Each core computes
attention for 8 heads of one batch plus the partial output projection over its
512 head-dims; the host sums the two partials per batch and adds b_proj.

Per-core kernel (fp16 data path, fp32 PSUM accumulation):
  1. x is cast to fp16 on the host; xT tiles arrive via XBAR DMA transpose
     straight from DRAM (no PE transposes, no PSUM->SBUF copies).
  2. qT/kT computed head-transposed ([dims, tokens], lhsT = w slice),
     bias-added into fp16; v computed natural ([tokens, dims]) with a ones
     column per (k-tile, head) group (v_aug) so the flipped PV matmul also
     yields the softmax denominator.
  3. S^T tiles [k=128, q=512] for the two heads of a pair via two
     row-group-packed matmuls into one 2-bank PSUM tile [128, 1024].
  4. E = exp(scale * S^T) on ScalarE straight out of PSUM into fp16.
  5. PV flipped: out[q, d] per (head, 128-query block): lhsT = E slice
     [128k, 128q], rhs = v_aug slice [128k, 65]; 65-column matmuls
     accumulate over k-tiles. PV for k-tile j is emitted after S^T of
     k-tile j+1 so the in-order PE queue never stalls on the exp.
  6. Tail: 2 reciprocals + 2 broadcast multiplies normalize into fp16 u
     tiles; XBAR DMA transpose writes them back as [dims, tokens] for the
     fp16 projection.
"""

import os
import sys

import numpy as np

try:
    import concourse.bass as bass
except ImportError:  # harness runs from a bare directory
    sys.path.insert(0, "/opt/trn_rl_repo")
    import concourse.bass as bass

import concourse.mybir as mybir
import concourse.tile as tile
from concourse.bass_utils import run_bass_kernel_spmd
from concourse.masks import make_identity

F32 = mybir.dt.float32
F16 = mybir.dt.float16
EXP = mybir.ActivationFunctionType.Exp
ADD = mybir.AluOpType.add
MULT = mybir.AluOpType.mult

B, N_FULL, D = 4, 2048, 1024
H, HD = 16, 64
NCORES = 8
GROUPS = 2          # head-groups (tensor parallel)
HL = H // GROUPS    # 8 heads per core
DL = HL * HD        # 512 local head-dims per core
PAIRS = HL // 2     # 4 head pairs
SCALE = HD ** -0.5
VG = HD + 1         # v dims + ones column per (k-tile, head)

LAST_EXEC_NS = None
EXPS = []  # debug: (p, qn, kt) per emitted exp, in ACT-stream order


def _split_multiwait_matmuls(raw: bytes) -> bytes:
    """This container's walrus allows at most one sync-wait per Matmult.

    Tile attaches up to 3. Hoist the extras onto standalone EventSemaphore
    instructions inserted immediately before the matmul on the same engine
    (identical semantics: the sequencer blocks on them in program order).
    """
    import json

    bir = json.loads(raw)
    n = [0]

    def fix_block(block):
        insts = block.get("instructions")
        if not isinstance(insts, list):
            return
        out = []
        for ins in insts:
            si = ins.get("sync_info") if isinstance(ins, dict) else None
            if (
                isinstance(ins, dict)
                and ins.get("opcode") != "EventSemaphore"
                and si
                and len(si.get("on_wait") or []) > 1
            ):
                waits = si["on_wait"]
                for w in waits[1:]:
                    n[0] += 1
                    out.append({
                        "debug": ins.get("debug", 0),
                        "engine": ins["engine"],
                        "ins": [],
                        "name": f"I-waitfix-{n[0]}",
                        "opcode": "EventSemaphore",
                        "outs": [],
                        "sync_info": {"on_update": [], "on_wait": [w]},
                    })
                si["on_wait"] = waits[:1]
            out.append(ins)
        block["instructions"] = out

    for fn in bir.get("functions", []):
        for block in fn.get("blocks", []):
            fix_block(block)
    return json.dumps(bir).encode()


def build(N=N_FULL):
    NK = N // 128   # k tiles of 128
    NQ = N // 512   # q tiles of 512
    NTT = N // 512  # token tiles of 512 for the qkv projection

    nc = bass.Bass("TRN2", target_bir_lowering=False)
    x = nc.dram_tensor("x", [N, D], F16, kind="ExternalInput")
    # [ii, otile(4 q-pairs then 4 k-pairs), io, 128] so each DMA slab is
    # contiguous per partition.
    wqk = nc.dram_tensor("wqk", [128, 8, 8, 128], F16, kind="ExternalInput")
    wv = nc.dram_tensor("wv", [128, 8, DL], F16, kind="ExternalInput")
    bqk = nc.dram_tensor("bqk", [128, 8], F32, kind="ExternalInput")
    bv = nc.dram_tensor("bv", [128, DL], F32, kind="ExternalInput")
    wproj = nc.dram_tensor("wproj", [128, PAIRS, D], F16, kind="ExternalInput")
    out = nc.dram_tensor("out", [N, D], F16, kind="ExternalOutput")
    # partial projection (pairs 0-1) of the last query tile; the host adds
    # it onto out[3*512:], letting most of the final proj leave the tail
    out2 = nc.dram_tensor("out2", [512, D], F16, kind="ExternalOutput")

    with tile.TileContext(nc) as tc:
        with (
            tc.tile_pool(name="const", bufs=1) as const_pool,
            tc.tile_pool(name="wres", bufs=1) as wres_pool,
            tc.tile_pool(name="wqs", bufs=3) as wqs_pool,
            tc.tile_pool(name="xt", bufs=4) as xt_pool,
            tc.tile_pool(name="qk", bufs=1) as qk_pool,
            tc.tile_pool(name="vg", bufs=1) as vg_pool,
            tc.tile_pool(name="at", bufs=2) as at_pool,
            tc.tile_pool(name="ep", bufs=4) as e_pool,
            tc.tile_pool(name="rp", bufs=2) as r_pool,
            tc.tile_pool(name="up", bufs=6) as u_pool,
            tc.tile_pool(name="sg", bufs=10) as sg_pool,
            tc.tile_pool(name="ob", bufs=3) as ob_pool,
            tc.tile_pool(name="psst", bufs=2, space="PSUM") as pss_pool,
            tc.tile_pool(name="pspv", bufs=2, space="PSUM") as psv_pool,
            tc.tile_pool(name="ps512", bufs=2, space="PSUM") as p5_pool,
        ):
            # Warm up the ACT exp table while the first DMAs are in flight so
            # the first real exp doesn't pay the table load.
            warm = const_pool.tile([128, 1], F32)
            nc.gpsimd.memset(warm[:, :], 0.0)
            nc.scalar.activation(warm[:, :], warm[:, :], EXP)

            bqk_sb = const_pool.tile([128, 8], F32)

            # fp16 identity for the PE transposes in the attention tail
            # (fp16 memset is ISA-invalid, so build in f32 and cast)
            ident32 = const_pool.tile([128, 128], F32)
            make_identity(nc, ident32[:, :])
            ident16 = const_pool.tile([128, 128], F16)
            nc.vector.tensor_copy(ident16[:, :], ident32[:, :])

            qT = qk_pool.tile([128, PAIRS, N], F16, tag="qT")
            kT = qk_pool.tile([128, PAIRS, N], F16, tag="kT")
            # Flat v layout: per (k-tile, head) a 65-column group = 64 v-dims
            # + ones column (PV denominator column after the flip).
            vaug = vg_pool.tile([128, NK * HL * VG], F16, tag="vaug")
            ones_view = vaug[:, :].rearrange(
                "p (g c) -> p g c", c=VG)[:, :, HD:HD + 1]
            nc.vector.tensor_scalar(
                out=ones_view, in0=warm[:, None, 0:1].broadcast_to(
                    [128, NK * HL, 1]),
                scalar1=0.0, scalar2=1.0, op0=MULT, op1=ADD,
            )

            class Chain:
                """One (pair, qn) attention chain, PV pipelined one kt back.

                Can be spilled mid-way: the PSUM partials move to SBUF
                segment tiles so another chain can use the PSUM banks, and
                segments are merged back in finish().
                """

                def __init__(self, p, qn):
                    self.p, self.qn = p, qn
                    self.pvA = self.pvB = None
                    self.segs = [None, None]
                    self.prev = None   # (e2, kt) awaiting its PV
                    self.first = True

                def _pvs(self):
                    return ((0, self.pvA), (1, self.pvB))

                def _pv(self):
                    e2, kt = self.prev
                    self.prev = None
                    for hh, pv in self._pvs():
                        vo = (kt * HL + 2 * self.p + hh) * VG
                        for qs in range(4):
                            nc.tensor.matmul(
                                pv[:, qs, :],
                                lhsT=e2[:, hh * 512 + qs * 128:
                                        hh * 512 + (qs + 1) * 128],
                                rhs=vaug[:, vo:vo + VG],
                                start=(self.first and qs == 0),
                                stop=False,
                                skip_group_check=True,
                            )
                        if hh == 1:
                            self.first = False

                def step(self, kt):
                    if self.pvA is None:
                        self.pvA = psv_pool.tile([128, 4, VG], F32, tag="pv",
                                                 name="pvA")
                        self.pvB = psv_pool.tile([128, 4, VG], F32, tag="pv",
                                                 name="pvB")
                        self.first = True
                    q0 = self.qn * 512
                    k0 = kt * 128
                    stab = pss_pool.tile([128, 1024], F32, tag="st",
                                         name="stab")
                    for fo, base in ((0, 0), (512, 64)):
                        nc.tensor.matmul(
                            stab[:, fo:fo + 512],
                            lhsT=kT[base:base + 64, self.p, k0:k0 + 128],
                            rhs=qT[base:base + 64, self.p, q0:q0 + 512],
                            start=True,
                            stop=True,
                            tile_position=(base, 0),
                            skip_group_check=True,
                        )
                    e2 = e_pool.tile([128, 1024], F16, tag="e", name="e2")
                    nc.scalar.activation(e2[:, :], stab[:, :], EXP, scale=SCALE)
                    EXPS.append((self.p, self.qn, kt))
                    if self.prev is not None:
                        self._pv()
                    self.prev = (e2, kt)

                def spill(self):
                    """Drain the pending PV and move partials to SBUF."""
                    if self.prev is not None:
                        self._pv()
                    for hh, pv in self._pvs():
                        if self.segs[hh] is None:
                            seg = sg_pool.tile([128, 4, VG], F32, tag="sg",
                                               name="seg")
                            nc.vector.tensor_copy(seg[:, :, :], pv[:, :, :])
                        else:
                            seg = sg_pool.tile([128, 4, VG], F32, tag="sg",
                                               name="seg")
                            nc.vector.tensor_tensor(
                                out=seg[:, :, :], in0=pv[:, :, :],
                                in1=self.segs[hh][:, :, :], op=ADD)
                        self.segs[hh] = seg
                    self.pvA = self.pvB = None

                def finish(self, at_t):
                    if self.prev is not None:
                        self._pv()
                    p = self.p
                    # merge spilled segments, then normalize by the
                    # per-query denominator (column 64) during the fp16 copy
                    rcs = r_pool.tile([128, 2, 4, 1], F32, tag="rc", name="rcs")
                    srcs = []
                    for hh, pv in self._pvs():
                        if self.segs[hh] is not None:
                            fin = sg_pool.tile([128, 4, VG], F32, tag="sg",
                                               name="fin")
                            nc.vector.tensor_tensor(
                                out=fin[:, :, :], in0=pv[:, :, :],
                                in1=self.segs[hh][:, :, :], op=ADD)
                            srcs.append(fin)
                        else:
                            srcs.append(pv)
                    for hh, src in enumerate(srcs):
                        nc.vector.reciprocal(
                            rcs[:, hh, :, :], src[:, :, HD:HD + 1])
                    tr = p5_pool.tile([128, 512], F32, tag="p512", name="tr")
                    for hh, src in enumerate(srcs):
                        u = u_pool.tile([128, 4, HD], F16, tag="u", name="u")
                        nc.vector.tensor_tensor(
                            out=u[:, :, :],
                            in0=src[:, :, 0:HD],
                            in1=rcs[:, hh, :, :].broadcast_to([128, 4, HD]),
                            op=MULT,
                        )
                        for qs in range(4):
                            nc.tensor.matmul(
                                tr[hh * 64:(hh + 1) * 64,
                                   qs * 128:(qs + 1) * 128],
                                lhsT=u[:, qs, :],
                                rhs=ident16[:, :],
                                start=True,
                                stop=True,
                                skip_group_check=True,
                            )
                    nc.vector.tensor_copy(at_t[:, p, :], tr[:, :])
                    self.pvA = self.pvB = None
                    self.segs = [None, None]

            # Early chain: (pair 0, qn 0) runs during the kv phase — its
            # k-tiles become valid t-tile by t-tile, so its exps fill the
            # otherwise ACT-idle prefix.
            at0 = at_pool.tile([128, PAIRS, 512], F16, tag="at", name="at0")

            xts = []

            def qk_quarter(ti, o, state, q, xt):
                """Two ics of a q/k projection block, for side-work
                pumping: keeps per-pump PE cost at ~0.43us."""
                if q == 0:
                    wo = wqs_pool.tile([128, 8, 128], F16, tag="wo")
                    nc.sync.dma_start(wo[:, :, :], wqk[:, o, :, :])
                    qp = p5_pool.tile([128, 512], F32, tag="p512", name="qp")
                    state[:] = [wo, qp]
                wo, qp = state
                for ic in range(q * 2, q * 2 + 2):
                    nc.tensor.matmul(
                        qp[:, :],
                        lhsT=wo[:, ic, :],
                        rhs=xt[:, ic, :],
                        start=(ic == 0),
                        stop=(ic == 7),
                    )
                if q == 3:
                    dst = qT if o < 4 else kT
                    nc.vector.tensor_scalar_add(
                        dst[:, o % 4, ti * 512:(ti + 1) * 512], qp[:, :],
                        bqk_sb[:, o:o + 1],
                    )

            def qk_side(ti, o):
                state = []
                return [lambda q=q: qk_quarter(ti, o, state, q, xts[ti])
                        for q in (0, 1, 2, 3)]

            def qk_group(ti, o, xt):
                """One 128-dim output block of the q/k projection."""
                wo = wqs_pool.tile([128, 8, 128], F16, tag="wo")
                nc.sync.dma_start(wo[:, :, :], wqk[:, o, :, :])
                qp = p5_pool.tile([128, 512], F32, tag="p512", name="qp")
                for ic in range(8):
                    nc.tensor.matmul(
                        qp[:, :],
                        lhsT=wo[:, ic, :],
                        rhs=xt[:, ic, :],
                        start=(ic == 0),
                        stop=(ic == 7),
                    )
                dst = qT if o < 4 else kT
                nc.vector.tensor_scalar_add(
                    dst[:, o % 4, ti * 512:(ti + 1) * 512], qp[:, :],
                    bqk_sb[:, o:o + 1],
                )

            # ---- phase 1: xT (DMA transpose from DRAM), v, kT for every
            # token tile, plus the qn0 queries; q projections for qn1..3 are
            # deferred into the ACT-bound attention stretch (JIT q). The qn0
            # chains run here against the PE-dense stretch, spilling their
            # PSUM partials so only one chain holds banks at a time.
            chains0 = [Chain(p, 0) for p in range(PAIRS)]
            chA, chB, chC, chD = chains0
            ch10 = Chain(0, 1)

            def v_group(ti, s, xt):
                r = ti * 4 + s
                vp = p5_pool.tile([128, DL], F32, tag="p512", name="vp")
                for ic in range(8):
                    nc.tensor.matmul(
                        vp[:, :],
                        lhsT=xt[:, ic, s * 128:(s + 1) * 128],
                        rhs=wv_sb[:, ic, :],
                        start=(ic == 0),
                        stop=(ic == 7),
                    )
                nc.vector.tensor_tensor(
                    out=vaug[:, r * HL * VG:(r + 1) * HL * VG].rearrange(
                        "p (h c) -> p h c", c=VG)[:, :, 0:HD],
                    in0=vp[:, :].rearrange("p (h d) -> p h d", h=HL),
                    in1=bv_sb[:, :].rearrange("p (h d) -> p h d", h=HL),
                    op=ADD,
                )

            for ti in range(NTT):
                xt = xt_pool.tile([128, 8, 512], F16, tag="xt",
                                  name=f"xt{ti}")
                xts.append(xt)
                if ti == 0:
                    # two half-tile transposes so the first q/k matmuls can
                    # start while the second half is still in flight; the
                    # first wo tiles are prefetched between them
                    nc.sync.dma_start_transpose(
                        xt[:, :, 0:256], x[0:256, :])
                    wos0 = {}
                    for o in (4, 0):
                        wo_pre = wqs_pool.tile([128, 8, 128], F16, tag="wo",
                                               name=f"wo_pre{o}")
                        nc.sync.dma_start(wo_pre[:, :, :], wqk[:, o, :, :])
                        wos0[o] = wo_pre
                    nc.sync.dma_start_transpose(
                        xt[:, :, 256:512], x[256:512, :])
                    nc.sync.dma_start(bqk_sb[:, :], bqk[:, :])
                else:
                    nc.sync.dma_start_transpose(
                        xt[:, :, :], x[ti * 512:(ti + 1) * 512, :])
                if ti == 0:
                    # queries/keys for pair 0 first so the exp stream starts
                    # as early as possible; chain B follows A inside ti0.
                    for o in (4, 0):
                        wo = wos0[o]
                        qp = p5_pool.tile([128, 512], F32, tag="p512",
                                          name="qp")
                        for half in (0, 1):
                            for ic in range(8):
                                nc.tensor.matmul(
                                    qp[:, half * 256:(half + 1) * 256],
                                    lhsT=wo[:, ic, :],
                                    rhs=xt[:, ic, half * 256:(half + 1) * 256],
                                    start=(half == 0 and ic == 0),
                                    stop=(half == 1 and ic == 7),
                                    skip_group_check=True,
                                )
                        dst = qT if o < 4 else kT
                        nc.vector.tensor_scalar_add(
                            dst[:, o % 4, 0:512], qp[:, :], bqk_sb[:, o:o + 1])
                    bv_sb = const_pool.tile([128, DL], F32)
                    nc.sync.dma_start(bv_sb[:, :], bv[:, :])
                    wv_sb = wres_pool.tile([128, 8, DL], F16)
                    nc.sync.dma_start(wv_sb[:, :, :], wv[:, :, :])
                    chA.step(0)
                    qk_group(0, 5, xt)
                    qk_group(0, 1, xt)
                    v_group(0, 0, xt)
                    chA.step(1)
                    v_group(0, 1, xt)
                    chA.step(2)
                    v_group(0, 2, xt)
                    chA.step(3)
                    v_group(0, 3, xt)
                    chB.step(0)
                    chA.spill()
                    chB.step(1)
                    chB.step(2)
                    chB.step(3)
                    chB.spill()
                    for o in (6, 2, 7, 3):
                        qk_group(0, o, xt)
                elif ti == 1:
                    qk_group(1, 6, xt)
                    chC.step(0)
                    chC.step(1)
                    v_group(1, 0, xt)
                    chC.step(2)
                    v_group(1, 1, xt)
                    chC.step(3)
                    v_group(1, 2, xt)
                    chC.step(4)
                    v_group(1, 3, xt)
                    chC.step(5)
                    chC.step(6)
                    chC.step(7)
                    qk_group(1, 7, xt)
                    chD.step(0)
                    chC.spill()
                    for kt in range(1, 8):
                        chD.step(kt)
                    chD.spill()
                    for o in (4, 5):
                        qk_group(1, o, xt)
                elif ti == 2:
                    qk_group(2, 4, xt)
                    chA.step(4)
                    chA.step(5)
                    v_group(2, 0, xt)
                    chA.step(6)
                    v_group(2, 1, xt)
                    chA.step(7)
                    v_group(2, 2, xt)
                    chA.step(8)
                    v_group(2, 3, xt)
                    chA.step(9)
                    chA.step(10)
                    chA.step(11)
                    qk_group(2, 5, xt)
                    chB.step(4)
                    chA.spill()
                    for kt in range(5, 12):
                        chB.step(kt)
                    chB.spill()
                    for o in (6, 7):
                        qk_group(2, o, xt)
                else:
                    qk_group(3, 6, xt)
                    chC.step(8)
                    chC.step(9)
                    v_group(3, 0, xt)
                    chC.step(10)
                    v_group(3, 1, xt)
                    chC.step(11)
                    v_group(3, 2, xt)
                    chC.step(12)
                    v_group(3, 3, xt)
                    chC.step(13)
                    chC.step(14)
                    chC.step(15)
                    qk_group(3, 7, xt)
                    chD.step(8)
                    chC.finish(at0)
                    for kt in range(9, 16):
                        chD.step(kt)
                    qk_group(3, 4, xt)
                    chA.step(12)
                    chD.finish(at0)
                    chA.step(13)
                    chA.step(14)
                    chA.step(15)
                    qk_group(3, 5, xt)
                    chB.step(12)
                    chA.finish(at0)
                    chB.step(13)
                    chB.step(14)
                    qk_group(1, 0, xts[1])
                    chB.step(15)
                    chB.finish(at0)
                    # first k-tiles of the next query tile's first chain:
                    # phase 2 is exp-bound, so every exp pulled into the
                    # PE-bound kv phase is nearly free
                    for kt in range(3):
                        ch10.step(kt)

            # w_proj is first read by the projection, deep into the
            # attention phase; loading it here keeps the head-of-queue DMA
            # slots for the x tiles the qkv matmuls are waiting on.
            wp_sb = wres_pool.tile([128, PAIRS, D], F16)
            nc.sync.dma_start(wp_sb[:, :, :], wproj[:, :, :])

            def proj(at_t, qn_t):
                for s in range(4):
                    t0 = qn_t * 512 + s * 128
                    for e in range(2):
                        op_ = p5_pool.tile([128, 512], F32, tag="p512", name="op")
                        for p_ in range(PAIRS):
                            nc.tensor.matmul(
                                op_[:, :],
                                lhsT=at_t[:, p_, s * 128:(s + 1) * 128],
                                rhs=wp_sb[:, p_, e * 512:(e + 1) * 512],
                                start=(p_ == 0),
                                stop=(p_ == PAIRS - 1),
                            )
                        ob = ob_pool.tile([128, 512], F16, tag="ob")
                        nc.vector.tensor_copy(ob[:, :], op_[:, :])
                        nc.sync.dma_start(
                            out[t0:t0 + 128, e * 512:(e + 1) * 512], ob[:, :])

            # ---- attention + projection, one 512-query tile at a time.
            # The stretch is ACT(exp)-bound; JIT q projections for qn+1,
            # proj(qn-1) matmul groups, and output copies are queued as side
            # work and pumped one item per odd k-tile into the PE-idle slack.
            side = []

            def pump():
                if side:
                    side.pop(0)()

            def proj_part(at_t, t0, s, e, state, half, dst=None):
                if half == 0:
                    state[:] = [p5_pool.tile([128, 512], F32, tag="p512",
                                             name="op")]
                op_ = state[0]
                for p_ in (half * 2, half * 2 + 1):
                    nc.tensor.matmul(
                        op_[:, :],
                        lhsT=at_t[:, p_, s * 128:(s + 1) * 128],
                        rhs=wp_sb[:, p_, e * 512:(e + 1) * 512],
                        start=(p_ == 0),
                        stop=(p_ == PAIRS - 1),
                    )
                if half == 1:
                    ob = ob_pool.tile([128, 512], F16, tag="ob")
                    nc.vector.tensor_copy(ob[:, :], op_[:, :])
                    d = out if dst is None else dst
                    nc.sync.dma_start(
                        d[t0:t0 + 128, e * 512:(e + 1) * 512], ob[:, :])

            def proj_side(at_t, qn_t):
                work = []
                for s in range(4):
                    t0 = qn_t * 512 + s * 128
                    for e in range(2):
                        state = []
                        work.extend(
                            lambda s=s, e=e, t0=t0, st=state, h=h:
                            proj_part(at_t, t0, s, e, st, h)
                            for h in (0, 1))
                return work

            def proj_group(at_t, t0, s, e, pairs=range(PAIRS), dst=None):
                op_ = p5_pool.tile([128, 512], F32, tag="p512", name="op")
                pl = list(pairs)
                for p_ in pl:
                    nc.tensor.matmul(
                        op_[:, :],
                        lhsT=at_t[:, p_, s * 128:(s + 1) * 128],
                        rhs=wp_sb[:, p_, e * 512:(e + 1) * 512],
                        start=(p_ == pl[0]),
                        stop=(p_ == pl[-1]),
                    )
                ob = ob_pool.tile([128, 512], F16, tag="ob")
                nc.vector.tensor_copy(ob[:, :], op_[:, :])
                d = out if dst is None else dst
                nc.sync.dma_start(
                    d[t0:t0 + 128, e * 512:(e + 1) * 512], ob[:, :])

            # Flat phase-2 plan over qn1..3; each chain's finish is
            # deferred until two steps into the next chain so the exp stream
            # never breaks at a chain boundary. JIT q projections and proj
            # output groups ride the side-work queue, pumped one item per
            # odd k-tile into the ACT-bound stretch's PE-idle slack.
            plan = []
            ats = {0: at0}
            for qn in range(1, NQ):
                ats[qn] = at_pool.tile([128, PAIRS, 512], F16, tag="at",
                                       name="at")
                plan += [((ch10, range(3, NK), 1) if (qn == 1 and p == 0)
                          else (Chain(p, qn), range(NK), qn))
                         for p in range(PAIRS)]

            for o in (1, 2, 3):
                side.extend(qk_side(1, o))
            side.extend(qk_side(2, 0))
            side.extend(proj_side(ats[0], 0))

            deferred = None
            for ch, kts, qn in plan:
                if qn == 2 and ch.p == 0:
                    for o in (1, 2, 3):
                        side.extend(qk_side(2, o))
                    side.extend(qk_side(3, 0))
                    side.extend(proj_side(ats[1], 1))
                if qn == 3 and ch.p == 0:
                    for o in (1, 2, 3):
                        side.extend(qk_side(3, o))
                    side.extend(proj_side(ats[2], 2))
                if qn == 3 and ch.p == 2:
                    side.extend(
                        lambda s=s, e=e: proj_group(
                            ats[3], s * 128, s, e, pairs=range(2), dst=out2)
                        for s in range(4) for e in range(2))
                    # (each item is a 2-matmul group + copy: ~0.43us PE)
                for idx, kt in enumerate(kts):
                    ch.step(kt)
                    if idx == 1 and deferred is not None:
                        dch, dqn = deferred
                        dch.finish(ats[dqn])
                        deferred = None
                    if idx >= 5:
                        pump()
                deferred = (ch, qn)
            dch, dqn = deferred
            dch.finish(ats[dqn])
            while side:
                pump()
            for s in range(4):
                for e in range(2):
                    op_ = p5_pool.tile([128, 512], F32, tag="p512", name="op")
                    for p_ in (2, 3):
                        nc.tensor.matmul(
                            op_[:, :],
                            lhsT=ats[3][:, p_, s * 128:(s + 1) * 128],
                            rhs=wp_sb[:, p_, e * 512:(e + 1) * 512],
                            start=(p_ == 2),
                            stop=(p_ == 3),
                        )
                    ob = ob_pool.tile([128, 512], F16, tag="obt",
                                      name="obt", bufs=4)
                    if e == 0:
                        nc.vector.tensor_copy(ob[:, :], op_[:, :])
                        eng = nc.sync
                    else:
                        nc.scalar.activation(
                            ob[:, :], op_[:, :],
                            mybir.ActivationFunctionType.Copy)
                        eng = nc.scalar
                    eng.dma_start(
                        out[3 * 512 + s * 128:3 * 512 + s * 128 + 128,
                            e * 512:(e + 1) * 512], ob[:, :])
    _orig_to_json = nc.to_json_bytes
    nc.to_json_bytes = lambda: _split_multiwait_matmuls(_orig_to_json())
    return nc


def shard_inputs(x, w_qkv, b_qkv, w_proj, N=N_FULL):
    """Build the 8 per-core input maps from full inputs."""
    x = np.ascontiguousarray(np.asarray(x, dtype=np.float32))
    w_qkv = np.asarray(w_qkv, dtype=np.float32)
    b_qkv = np.asarray(b_qkv, dtype=np.float32)
    w_proj = np.asarray(w_proj, dtype=np.float32)
    in_maps = []
    for c in range(NCORES):
        b, g = divmod(c, 2)
        qc = slice(g * DL, (g + 1) * DL)
        wq = w_qkv[:, 0 * D:1 * D][:, qc]
        wk = w_qkv[:, 1 * D:2 * D][:, qc]
        wv_ = w_qkv[:, 2 * D:3 * D][:, qc]
        wqk_np = np.empty((128, 8, 8, 128), np.float32)
        bqk_np = np.empty((128, 8), np.float32)
        for o in range(8):
            src = wq if o < 4 else wk
            bsrc = b_qkv[0:D][qc] if o < 4 else b_qkv[D:2 * D][qc]
            blk = src[:, (o % 4) * 128:(o % 4 + 1) * 128].reshape(8, 128, 128)
            wqk_np[:, o] = blk.transpose(1, 0, 2)
            bqk_np[:, o] = bsrc[(o % 4) * 128:(o % 4 + 1) * 128]
        wv_np = np.ascontiguousarray(wv_.reshape(8, 128, DL).transpose(1, 0, 2))
        bv_np = np.broadcast_to(b_qkv[2 * D:3 * D][qc], (128, DL)).copy()
        wp_np = np.ascontiguousarray(
            w_proj[g * DL:(g + 1) * DL, :].reshape(PAIRS, 128, D).transpose(1, 0, 2)
        )
        xb = x[min(b, x.shape[0] - 1), :N] if x.ndim == 3 else x[:N]
        in_maps.append({
            "x": np.ascontiguousarray(xb).astype(np.float16),
            "wqk": wqk_np.astype(np.float16),
            "wv": wv_np.astype(np.float16),
            "bqk": bqk_np,
            "bv": bv_np,
            "wproj": wp_np.astype(np.float16),
        })
    return in_maps


_NC_CACHE = {}


def kernel(x, w_qkv, b_qkv, w_proj, b_proj):
    global LAST_EXEC_NS
    x = np.asarray(x, dtype=np.float32)
    b_proj = np.asarray(b_proj, dtype=np.float32)
    if N_FULL not in _NC_CACHE:
        _NC_CACHE[N_FULL] = build(N_FULL)
    nc = _NC_CACHE[N_FULL]
    in_maps = shard_inputs(x, w_qkv, b_qkv, w_proj)
    trace = os.environ.get("KERNEL_TRACE", "0") == "1"
    res = run_bass_kernel_spmd(
        nc, in_maps, core_ids=list(range(NCORES)), trace=trace,
        trace_cores=[0] if trace else None,
    )
    LAST_EXEC_NS = res.exec_time_ns
    full = np.empty((B, N_FULL, D), np.float32)
    for b in range(B):
        r0, r1 = res.results[2 * b], res.results[2 * b + 1]
        full[b] = r0["out"].astype(np.float32) + r1["out"].astype(np.float32)
        full[b][3 * 512:] += (r0["out2"].astype(np.float32)
                              + r1["out2"].astype(np.float32))
    full += b_proj[None, None, :]
    return full


# revision 8
# speedup vs baseline: 1.3004x; 1.0040x over previous
"""Multi-head attention block (B=4, N=2048, D=1024, H=16) on 8 trn2 NeuronCores.

Sharding: core c -> (batch b = c//2, head-group g = c%2). Each core computes
attention for 8 heads of one batch plus the partial output projection over its
512 head-dims; the host sums the two partials per batch and adds b_proj.

Per-core kernel (fp16 data path, fp32 PSUM accumulation):
  1. x is cast to fp16 on the host; xT tiles arrive via XBAR DMA transpose
     straight from DRAM (no PE transposes, no PSUM->SBUF copies).
  2. qT/kT computed head-transposed ([dims, tokens], lhsT = w slice),
     bias-added into fp16; v computed natural ([tokens, dims]) with a ones
     column per (k-tile, head) group (v_aug) so the flipped PV matmul also
     yields the softmax denominator.
  3. S^T tiles [k=128, q=512] for the two heads of a pair via two
     row-group-packed matmuls into one 2-bank PSUM tile [128, 1024].
  4. E = exp(scale * S^T) on ScalarE straight out of PSUM into fp16.
  5. PV flipped: out[q, d] per (head, 128-query block): lhsT = E slice
     [128k, 128q], rhs = v_aug slice [128k, 65]; 65-column matmuls
     accumulate over k-tiles. PV for k-tile j is emitted after S^T of
     k-tile j+1 so the in-order PE queue never stalls on the exp.
  6. Tail: 2 reciprocals + 2 broadcast multiplies normalize into fp16 u
     tiles; XBAR DMA transpose writes them back as [dims, tokens] for the
     fp16 projection.
"""

import os
import sys

import numpy as np

try:
    import concourse.bass as bass
except ImportError:  # harness runs from a bare directory
    sys.path.insert(0, "/opt/trn_rl_repo")
    import concourse.bass as bass

import concourse.mybir as mybir
import concourse.tile as tile
from concourse.bass_utils import run_bass_kernel_spmd
from concourse.masks import make_identity

F32 = mybir.dt.float32
F16 = mybir.dt.float16
EXP = mybir.ActivationFunctionType.Exp
ADD = mybir.AluOpType.add
MULT = mybir.AluOpType.mult

B, N_FULL, D = 4, 2048, 1024
H, HD = 16, 64
NCORES = 8
GROUPS = 2          # head-groups (tensor parallel)
HL = H // GROUPS    # 8 heads per core
DL = HL * HD        # 512 local head-dims per core
PAIRS = HL // 2     # 4 head pairs
SCALE = HD ** -0.5
VG = HD + 1         # v dims + ones column per (k-tile, head)

LAST_EXEC_NS = None
EXPS = []  # debug: (p, qn, kt) per emitted exp, in ACT-stream order


def _split_multiwait_matmuls(raw: bytes) -> bytes:
    """This container's walrus allows at most one sync-wait per Matmult.

    Tile attaches up to 3. Hoist the extras onto standalone EventSemaphore
    instructions inserted immediately before the matmul on the same engine
    (identical semantics: the sequencer blocks on them in program order).
    """
    import json

    bir = json.loads(raw)
    n = [0]

    def fix_block(block):
        insts = block.get("instructions")
        if not isinstance(insts, list):
            return
        out = []
        for ins in insts:
            si = ins.get("sync_info") if isinstance(ins, dict) else None
            if (
                isinstance(ins, dict)
                and ins.get("opcode") != "EventSemaphore"
                and si
                and len(si.get("on_wait") or []) > 1
            ):
                waits = si["on_wait"]
                for w in waits[1:]:
                    n[0] += 1
                    out.append({
                        "debug": ins.get("debug", 0),
                        "engine": ins["engine"],
                        "ins": [],
                        "name": f"I-waitfix-{n[0]}",
                        "opcode": "EventSemaphore",
                        "outs": [],
                        "sync_info": {"on_update": [], "on_wait": [w]},
                    })
                si["on_wait"] = waits[:1]
            out.append(ins)
        block["instructions"] = out

    for fn in bir.get("functions", []):
        for block in fn.get("blocks", []):
            fix_block(block)
    return json.dumps(bir).encode()


def build(N=N_FULL):
    NK = N // 128   # k tiles of 128
    NQ = N // 512   # q tiles of 512
    NTT = N // 512  # token tiles of 512 for the qkv projection

    nc = bass.Bass("TRN2", target_bir_lowering=False)
    x = nc.dram_tensor("x", [N, D], F16, kind="ExternalInput")
    # [ii, otile(4 q-pairs then 4 k-pairs), io, 128] so each DMA slab is
    # contiguous per partition.
    wqk = nc.dram_tensor("wqk", [128, 8, 8, 128], F16, kind="ExternalInput")
    wv = nc.dram_tensor("wv", [128, 8, DL], F16, kind="ExternalInput")
    bqk = nc.dram_tensor("bqk", [128, 8], F32, kind="ExternalInput")
    bv = nc.dram_tensor("bv", [128, DL], F32, kind="ExternalInput")
    wproj = nc.dram_tensor("wproj", [128, PAIRS, D], F16, kind="ExternalInput")
    out = nc.dram_tensor("out", [N, D], F16, kind="ExternalOutput")
    # partial projection (pairs 0-1) of the last query tile; the host adds
    # it onto out[3*512:], letting most of the final proj leave the tail
    out2 = nc.dram_tensor("out2", [512, D], F16, kind="ExternalOutput")

    with tile.TileContext(nc) as tc:
        with (
            tc.tile_pool(name="const", bufs=1) as const_pool,
            tc.tile_pool(name="wres", bufs=1) as wres_pool,
            tc.tile_pool(name="wqs", bufs=3) as wqs_pool,
            tc.tile_pool(name="xt", bufs=4) as xt_pool,
            tc.tile_pool(name="qk", bufs=1) as qk_pool,
            tc.tile_pool(name="vg", bufs=1) as vg_pool,
            tc.tile_pool(name="at", bufs=2) as at_pool,
            tc.tile_pool(name="ep", bufs=4) as e_pool,
            tc.tile_pool(name="rp", bufs=2) as r_pool,
            tc.tile_pool(name="up", bufs=6) as u_pool,
            tc.tile_pool(name="sg", bufs=10) as sg_pool,
            tc.tile_pool(name="ob", bufs=3) as ob_pool,
            tc.tile_pool(name="psst", bufs=2, space="PSUM") as pss_pool,
            tc.tile_pool(name="pspv", bufs=2, space="PSUM") as psv_pool,
            tc.tile_pool(name="ps512", bufs=2, space="PSUM") as p5_pool,
        ):
            # Warm up the ACT exp table while the first DMAs are in flight so
            # the first real exp doesn't pay the table load.
            warm = const_pool.tile([128, 1], F32)
            nc.gpsimd.memset(warm[:, :], 0.0)
            nc.scalar.activation(warm[:, :], warm[:, :], EXP)

            bqk_sb = const_pool.tile([128, 8], F32)

            # fp16 identity for the PE transposes in the attention tail
            # (fp16 memset is ISA-invalid, so build in f32 and cast)
            ident32 = const_pool.tile([128, 128], F32)
            make_identity(nc, ident32[:, :])
            ident16 = const_pool.tile([128, 128], F16)
            nc.vector.tensor_copy(ident16[:, :], ident32[:, :])

            qT = qk_pool.tile([128, PAIRS, N], F16, tag="qT")
            kT = qk_pool.tile([128, PAIRS, N], F16, tag="kT")
            # Flat v layout: per (k-tile, head) a 65-column group = 64 v-dims
            # + ones column (PV denominator column after the flip).
            vaug = vg_pool.tile([128, NK * HL * VG], F16, tag="vaug")
            ones_view = vaug[:, :].rearrange(
                "p (g c) -> p g c", c=VG)[:, :, HD:HD + 1]
            nc.vector.tensor_scalar(
                out=ones_view, in0=warm[:, None, 0:1].broadcast_to(
                    [128, NK * HL, 1]),
                scalar1=0.0, scalar2=1.0, op0=MULT, op1=ADD,
            )

            class Chain:
                """One (pair, qn) attention chain, PV pipelined one kt back.

                Can be spilled mid-way: the PSUM partials move to SBUF
                segment tiles so another chain can use the PSUM banks, and
                segments are merged back in finish().
                """

                def __init__(self, p, qn):
                    self.p, self.qn = p, qn
                    self.pvA = self.pvB = None
                    self.segs = [None, None]
                    self.prev = None   # (e2, kt) awaiting its PV
                    self.first = True

                def _pvs(self):
                    return ((0, self.pvA), (1, self.pvB))

                def _pv(self):
                    e2, kt = self.prev
                    self.prev = None
                    for hh, pv in self._pvs():
                        vo = (kt * HL + 2 * self.p + hh) * VG
                        for qs in range(4):
                            nc.tensor.matmul(
                                pv[:, qs, :],
                                lhsT=e2[:, hh * 512 + qs * 128:
                                        hh * 512 + (qs + 1) * 128],
                                rhs=vaug[:, vo:vo + VG],
                                start=(self.first and qs == 0),
                                stop=False,
                                skip_group_check=True,
                            )
                        if hh == 1:
                            self.first = False

                def step(self, kt):
                    if self.pvA is None:
                        self.pvA = psv_pool.tile([128, 4, VG], F32, tag="pv",
                                                 name="pvA")
                        self.pvB = psv_pool.tile([128, 4, VG], F32, tag="pv",
                                                 name="pvB")
                        self.first = True
                    q0 = self.qn * 512
                    k0 = kt * 128
                    stab = pss_pool.tile([128, 1024], F32, tag="st",
                                         name="stab")
                    for fo, base in ((0, 0), (512, 64)):
                        nc.tensor.matmul(
                            stab[:, fo:fo + 512],
                            lhsT=kT[base:base + 64, self.p, k0:k0 + 128],
                            rhs=qT[base:base + 64, self.p, q0:q0 + 512],
                            start=True,
                            stop=True,
                            tile_position=(base, 0),
                            skip_group_check=True,
                        )
                    e2 = e_pool.tile([128, 1024], F16, tag="e", name="e2")
                    nc.scalar.activation(e2[:, :], stab[:, :], EXP, scale=SCALE)
                    EXPS.append((self.p, self.qn, kt))
                    if self.prev is not None:
                        self._pv()
                    self.prev = (e2, kt)

                def spill(self):
                    """Drain the pending PV and move partials to SBUF."""
                    if self.prev is not None:
                        self._pv()
                    for hh, pv in self._pvs():
                        if self.segs[hh] is None:
                            seg = sg_pool.tile([128, 4, VG], F32, tag="sg",
                                               name="seg")
                            nc.vector.tensor_copy(seg[:, :, :], pv[:, :, :])
                        else:
                            seg = sg_pool.tile([128, 4, VG], F32, tag="sg",
                                               name="seg")
                            nc.vector.tensor_tensor(
                                out=seg[:, :, :], in0=pv[:, :, :],
                                in1=self.segs[hh][:, :, :], op=ADD)
                        self.segs[hh] = seg
                    self.pvA = self.pvB = None

                def finish(self, at_t):
                    if self.prev is not None:
                        self._pv()
                    p = self.p
                    # merge spilled segments, then normalize by the
                    # per-query denominator (column 64) during the fp16 copy
                    rcs = r_pool.tile([128, 2, 4, 1], F32, tag="rc", name="rcs")
                    srcs = []
                    for hh, pv in self._pvs():
                        if self.segs[hh] is not None:
                            fin = sg_pool.tile([128, 4, VG], F32, tag="sg",
                                               name="fin")
                            nc.vector.tensor_tensor(
                                out=fin[:, :, :], in0=pv[:, :, :],
                                in1=self.segs[hh][:, :, :], op=ADD)
                            srcs.append(fin)
                        else:
                            srcs.append(pv)
                    for hh, src in enumerate(srcs):
                        nc.vector.reciprocal(
                            rcs[:, hh, :, :], src[:, :, HD:HD + 1])
                    tr = p5_pool.tile([128, 512], F32, tag="p512", name="tr")
                    for hh, src in enumerate(srcs):
                        u = u_pool.tile([128, 4, HD], F16, tag="u", name="u")
                        nc.vector.tensor_tensor(
                            out=u[:, :, :],
                            in0=src[:, :, 0:HD],
                            in1=rcs[:, hh, :, :].broadcast_to([128, 4, HD]),
                            op=MULT,
                        )
                        for qs in range(4):
                            nc.tensor.matmul(
                                tr[hh * 64:(hh + 1) * 64,
                                   qs * 128:(qs + 1) * 128],
                                lhsT=u[:, qs, :],
                                rhs=ident16[:, :],
                                start=True,
                                stop=True,
                                skip_group_check=True,
                            )
                    nc.vector.tensor_copy(at_t[:, p, :], tr[:, :])
                    self.pvA = self.pvB = None
                    self.segs = [None, None]

            # Early chain: (pair 0, qn 0) runs during the kv phase — its
            # k-tiles become valid t-tile by t-tile, so its exps fill the
            # otherwise ACT-idle prefix.
            at0 = at_pool.tile([128, PAIRS, 512], F16, tag="at", name="at0")

            xts = []

            def qk_quarter(ti, o, state, q, xt):
                """Two ics of a q/k projection block, for side-work
                pumping: keeps per-pump PE cost at ~0.43us."""
                if q == 0:
                    wo = wqs_pool.tile([128, 8, 128], F16, tag="wo")
                    nc.sync.dma_start(wo[:, :, :], wqk[:, o, :, :])
                    qp = p5_pool.tile([128, 512], F32, tag="p512", name="qp")
                    state[:] = [wo, qp]
                wo, qp = state
                for ic in range(q * 2, q * 2 + 2):
                    nc.tensor.matmul(
                        qp[:, :],
                        lhsT=wo[:, ic, :],
                        rhs=xt[:, ic, :],
                        start=(ic == 0),
                        stop=(ic == 7),
                    )
                if q == 3:
                    dst = qT if o < 4 else kT
                    nc.vector.tensor_scalar_add(
                        dst[:, o % 4, ti * 512:(ti + 1) * 512], qp[:, :],
                        bqk_sb[:, o:o + 1],
                    )

            def qk_side(ti, o):
                state = []
                return [lambda q=q: qk_quarter(ti, o, state, q, xts[ti])
                        for q in (0, 1, 2, 3)]

            def qk_group(ti, o, xt):
                """One 128-dim output block of the q/k projection."""
                wo = wqs_pool.tile([128, 8, 128], F16, tag="wo")
                nc.sync.dma_start(wo[:, :, :], wqk[:, o, :, :])
                qp = p5_pool.tile([128, 512], F32, tag="p512", name="qp")
                for ic in range(8):
                    nc.tensor.matmul(
                        qp[:, :],
                        lhsT=wo[:, ic, :],
                        rhs=xt[:, ic, :],
                        start=(ic == 0),
                        stop=(ic == 7),
                    )
                dst = qT if o < 4 else kT
                nc.vector.tensor_scalar_add(
                    dst[:, o % 4, ti * 512:(ti + 1) * 512], qp[:, :],
                    bqk_sb[:, o:o + 1],
                )

            # ---- phase 1: xT (DMA transpose from DRAM), v, kT for every
            # token tile, plus the qn0 queries; q projections for qn1..3 are
            # deferred into the ACT-bound attention stretch (JIT q). The qn0
            # chains run here against the PE-dense stretch, spilling their
            # PSUM partials so only one chain holds banks at a time.
            chains0 = [Chain(p, 0) for p in range(PAIRS)]
            chA, chB, chC, chD = chains0
            ch10 = Chain(0, 1)

            def v_group(ti, s, xt):
                r = ti * 4 + s
                vp = p5_pool.tile([128, DL], F32, tag="p512", name="vp")
                for ic in range(8):
                    nc.tensor.matmul(
                        vp[:, :],
                        lhsT=xt[:, ic, s * 128:(s + 1) * 128],
                        rhs=wv_sb[:, ic, :],
                        start=(ic == 0),
                        stop=(ic == 7),
                    )
                nc.vector.tensor_tensor(
                    out=vaug[:, r * HL * VG:(r + 1) * HL * VG].rearrange(
                        "p (h c) -> p h c", c=VG)[:, :, 0:HD],
                    in0=vp[:, :].rearrange("p (h d) -> p h d", h=HL),
                    in1=bv_sb[:, :].rearrange("p (h d) -> p h d", h=HL),
                    op=ADD,
                )

            for ti in range(NTT):
                xt = xt_pool.tile([128, 8, 512], F16, tag="xt",
                                  name=f"xt{ti}")
                xts.append(xt)
                if ti == 0:
                    # two half-tile transposes so the first q/k matmuls can
                    # start while the second half is still in flight; the
                    # first wo tiles are prefetched between them
                    nc.sync.dma_start_transpose(
                        xt[:, :, 0:256], x[0:256, :])
                    wos0 = {}
                    for o in (4, 0):
                        wo_pre = wqs_pool.tile([128, 8, 128], F16, tag="wo",
                                               name=f"wo_pre{o}")
                        nc.sync.dma_start(wo_pre[:, :, :], wqk[:, o, :, :])
                        wos0[o] = wo_pre
                    nc.sync.dma_start_transpose(
                        xt[:, :, 256:512], x[256:512, :])
                    nc.sync.dma_start(bqk_sb[:, :], bqk[:, :])
                else:
                    nc.sync.dma_start_transpose(
                        xt[:, :, :], x[ti * 512:(ti + 1) * 512, :])
                if ti == 0:
                    # queries/keys for pair 0 first so the exp stream starts
                    # as early as possible; chain B follows A inside ti0.
                    for o in (4, 0):
                        wo = wos0[o]
                        qp = p5_pool.tile([128, 512], F32, tag="p512",
                                          name="qp")
                        for half in (0, 1):
                            for ic in range(8):
                                nc.tensor.matmul(
                                    qp[:, half * 256:(half + 1) * 256],
                                    lhsT=wo[:, ic, :],
                                    rhs=xt[:, ic, half * 256:(half + 1) * 256],
                                    start=(half == 0 and ic == 0),
                                    stop=(half == 1 and ic == 7),
                                    skip_group_check=True,
                                )
                        dst = qT if o < 4 else kT
                        nc.vector.tensor_scalar_add(
                            dst[:, o % 4, 0:512], qp[:, :], bqk_sb[:, o:o + 1])
                    bv_sb = const_pool.tile([128, DL], F32)
                    nc.sync.dma_start(bv_sb[:, :], bv[:, :])
                    wv_sb = wres_pool.tile([128, 8, DL], F16)
                    nc.sync.dma_start(wv_sb[:, :, :], wv[:, :, :])
                    chA.step(0)
                    v_group(0, 0, xt)
                    chA.step(1)
                    qk_group(0, 5, xt)
                    v_group(0, 1, xt)
                    chA.step(2)
                    qk_group(0, 1, xt)
                    v_group(0, 2, xt)
                    chA.step(3)
                    v_group(0, 3, xt)
                    chB.step(0)
                    chA.spill()
                    chB.step(1)
                    chB.step(2)
                    chB.step(3)
                    chB.spill()
                    for o in (6, 2, 7, 3):
                        qk_group(0, o, xt)
                elif ti == 1:
                    qk_group(1, 6, xt)
                    chC.step(0)
                    chC.step(1)
                    v_group(1, 0, xt)
                    chC.step(2)
                    v_group(1, 1, xt)
                    chC.step(3)
                    v_group(1, 2, xt)
                    chC.step(4)
                    v_group(1, 3, xt)
                    chC.step(5)
                    chC.step(6)
                    chC.step(7)
                    qk_group(1, 7, xt)
                    chD.step(0)
                    chC.spill()
                    for kt in range(1, 8):
                        chD.step(kt)
                    chD.spill()
                    qk_group(1, 4, xt)
                    qk_group(1, 5, xt)
                elif ti == 2:
                    qk_group(2, 4, xt)
                    chA.step(4)
                    chA.step(5)
                    v_group(2, 0, xt)
                    chA.step(6)
                    v_group(2, 1, xt)
                    chA.step(7)
                    v_group(2, 2, xt)
                    chA.step(8)
                    v_group(2, 3, xt)
                    chA.step(9)
                    chA.step(10)
                    chA.step(11)
                    qk_group(2, 5, xt)
                    chB.step(4)
                    chA.spill()
                    for kt in range(5, 8):
                        chB.step(kt)
                    qk_group(2, 6, xt)
                    for kt in range(8, 12):
                        chB.step(kt)
                    chB.spill()
                    qk_group(2, 7, xt)
                else:
                    qk_group(3, 6, xt)
                    chC.step(8)
                    chC.step(9)
                    v_group(3, 0, xt)
                    chC.step(10)
                    v_group(3, 1, xt)
                    chC.step(11)
                    v_group(3, 2, xt)
                    chC.step(12)
                    v_group(3, 3, xt)
                    chC.step(13)
                    chC.step(14)
                    chC.step(15)
                    qk_group(3, 7, xt)
                    chD.step(8)
                    chC.finish(at0)
                    for kt in range(9, 16):
                        chD.step(kt)
                    qk_group(3, 4, xt)
                    chA.step(12)
                    qk_group(3, 5, xt)
                    chD.finish(at0)
                    chA.step(13)
                    chA.step(14)
                    chA.step(15)
                    chB.step(12)
                    chA.finish(at0)
                    chB.step(13)
                    chB.step(14)
                    qk_group(1, 0, xts[1])
                    chB.step(15)
                    chB.finish(at0)
                    # first k-tiles of the next query tile's first chain:
                    # phase 2 is exp-bound, so every exp pulled into the
                    # PE-bound kv phase is nearly free
                    for kt in range(3):
                        ch10.step(kt)

            # w_proj is first read by the projection, deep into the
            # attention phase; loading it here keeps the head-of-queue DMA
            # slots for the x tiles the qkv matmuls are waiting on.
            wp_sb = wres_pool.tile([128, PAIRS, D], F16)
            nc.sync.dma_start(wp_sb[:, :, :], wproj[:, :, :])

            def proj(at_t, qn_t):
                for s in range(4):
                    t0 = qn_t * 512 + s * 128
                    for e in range(2):
                        op_ = p5_pool.tile([128, 512], F32, tag="p512", name="op")
                        for p_ in range(PAIRS):
                            nc.tensor.matmul(
                                op_[:, :],
                                lhsT=at_t[:, p_, s * 128:(s + 1) * 128],
                                rhs=wp_sb[:, p_, e * 512:(e + 1) * 512],
                                start=(p_ == 0),
                                stop=(p_ == PAIRS - 1),
                            )
                        ob = ob_pool.tile([128, 512], F16, tag="ob")
                        nc.vector.tensor_copy(ob[:, :], op_[:, :])
                        nc.sync.dma_start(
                            out[t0:t0 + 128, e * 512:(e + 1) * 512], ob[:, :])

            # ---- attention + projection, one 512-query tile at a time.
            # The stretch is ACT(exp)-bound; JIT q projections for qn+1,
            # proj(qn-1) matmul groups, and output copies are queued as side
            # work and pumped one item per odd k-tile into the PE-idle slack.
            side = []

            def pump():
                if side:
                    side.pop(0)()

            def proj_part(at_t, t0, s, e, state, half, dst=None):
                if half == 0:
                    state[:] = [p5_pool.tile([128, 512], F32, tag="p512",
                                             name="op")]
                op_ = state[0]
                for p_ in (half * 2, half * 2 + 1):
                    nc.tensor.matmul(
                        op_[:, :],
                        lhsT=at_t[:, p_, s * 128:(s + 1) * 128],
                        rhs=wp_sb[:, p_, e * 512:(e + 1) * 512],
                        start=(p_ == 0),
                        stop=(p_ == PAIRS - 1),
                    )
                if half == 1:
                    ob = ob_pool.tile([128, 512], F16, tag="ob")
                    nc.vector.tensor_copy(ob[:, :], op_[:, :])
                    d = out if dst is None else dst
                    nc.sync.dma_start(
                        d[t0:t0 + 128, e * 512:(e + 1) * 512], ob[:, :])

            def proj_side(at_t, qn_t):
                work = []
                for s in range(4):
                    t0 = qn_t * 512 + s * 128
                    for e in range(2):
                        state = []
                        work.extend(
                            lambda s=s, e=e, t0=t0, st=state, h=h:
                            proj_part(at_t, t0, s, e, st, h)
                            for h in (0, 1))
                return work

            def proj_group(at_t, t0, s, e, pairs=range(PAIRS), dst=None):
                op_ = p5_pool.tile([128, 512], F32, tag="p512", name="op")
                pl = list(pairs)
                for p_ in pl:
                    nc.tensor.matmul(
                        op_[:, :],
                        lhsT=at_t[:, p_, s * 128:(s + 1) * 128],
                        rhs=wp_sb[:, p_, e * 512:(e + 1) * 512],
                        start=(p_ == pl[0]),
                        stop=(p_ == pl[-1]),
                    )
                ob = ob_pool.tile([128, 512], F16, tag="ob")
                nc.vector.tensor_copy(ob[:, :], op_[:, :])
                d = out if dst is None else dst
                nc.sync.dma_start(
                    d[t0:t0 + 128, e * 512:(e + 1) * 512], ob[:, :])

            # Flat phase-2 plan over qn1..3; each chain's finish is
            # deferred until two steps into the next chain so the exp stream
            # never breaks at a chain boundary. JIT q projections and proj
            # output groups ride the side-work queue, pumped one item per
            # odd k-tile into the ACT-bound stretch's PE-idle slack.
            plan = []
            ats = {0: at0}
            for qn in range(1, NQ):
                ats[qn] = at_pool.tile([128, PAIRS, 512], F16, tag="at",
                                       name="at")
                plan += [((ch10, range(3, NK), 1) if (qn == 1 and p == 0)
                          else (Chain(p, qn), range(NK), qn))
                         for p in range(PAIRS)]

            for o in (1, 2, 3):
                side.extend(qk_side(1, o))
            side.extend(qk_side(2, 0))
            side.extend(proj_side(ats[0], 0))

            deferred = None
            for ch, kts, qn in plan:
                if qn == 2 and ch.p == 0:
                    for o in (1, 2, 3):
                        side.extend(qk_side(2, o))
                    side.extend(qk_side(3, 0))
                    side.extend(proj_side(ats[1], 1))
                if qn == 3 and ch.p == 0:
                    for o in (1, 2, 3):
                        side.extend(qk_side(3, o))
                    side.extend(proj_side(ats[2], 2))
                if qn == 3 and ch.p == 2:
                    side.extend(
                        lambda s=s, e=e: proj_group(
                            ats[3], s * 128, s, e, pairs=range(2), dst=out2)
                        for s in range(4) for e in range(2))
                    # (each item is a 2-matmul group + copy: ~0.43us PE)
                for idx, kt in enumerate(kts):
                    ch.step(kt)
                    if idx == 1 and deferred is not None:
                        dch, dqn = deferred
                        dch.finish(ats[dqn])
                        deferred = None
                    if idx >= 5:
                        pump()
                deferred = (ch, qn)
            dch, dqn = deferred
            dch.finish(ats[dqn])
            while side:
                pump()
            for s in range(4):
                for e in range(2):
                    op_ = p5_pool.tile([128, 512], F32, tag="p512", name="op")
                    for p_ in (2, 3):
                        nc.tensor.matmul(
                            op_[:, :],
                            lhsT=ats[3][:, p_, s * 128:(s + 1) * 128],
                            rhs=wp_sb[:, p_, e * 512:(e + 1) * 512],
                            start=(p_ == 2),
                            stop=(p_ == 3),
                        )
                    ob = ob_pool.tile([128, 512], F16, tag="obt",
                                      name="obt", bufs=4)
                    if e == 0:
                        nc.vector.tensor_copy(ob[:, :], op_[:, :])
                        eng = nc.sync
                    else:
                        nc.scalar.activation(
                            ob[:, :], op_[:, :],
                            mybir.ActivationFunctionType.Copy)
                        eng = nc.scalar
                    eng.dma_start(
                        out[3 * 512 + s * 128:3 * 512 + s * 128 + 128,
                            e * 512:(e + 1) * 512], ob[:, :])
    _orig_to_json = nc.to_json_bytes
    nc.to_json_bytes = lambda: _split_multiwait_matmuls(_orig_to_json())
    return nc


def shard_inputs(x, w_qkv, b_qkv, w_proj, N=N_FULL):
    """Build the 8 per-core input maps from full inputs."""
    x = np.ascontiguousarray(np.asarray(x, dtype=np.float32))
    w_qkv = np.asarray(w_qkv, dtype=np.float32)
    b_qkv = np.asarray(b_qkv, dtype=np.float32)
    w_proj = np.asarray(w_proj, dtype=np.float32)
    in_maps = []
    for c in range(NCORES):
        b, g = divmod(c, 2)
        qc = slice(g * DL, (g + 1) * DL)
        wq = w_qkv[:, 0 * D:1 * D][:, qc]
        wk = w_qkv[:, 1 * D:2 * D][:, qc]
        wv_ = w_qkv[:, 2 * D:3 * D][:, qc]
        wqk_np = np.empty((128, 8, 8, 128), np.float32)
        bqk_np = np.empty((128, 8), np.float32)
        for o in range(8):
            src = wq if o < 4 else wk
            bsrc = b_qkv[0:D][qc] if o < 4 else b_qkv[D:2 * D][qc]
            blk = src[:, (o % 4) * 128:(o % 4 + 1) * 128].reshape(8, 128, 128)
            wqk_np[:, o] = blk.transpose(1, 0, 2)
            bqk_np[:, o] = bsrc[(o % 4) * 128:(o % 4 + 1) * 128]
        wv_np = np.ascontiguousarray(wv_.reshape(8, 128, DL).transpose(1, 0, 2))
        bv_np = np.broadcast_to(b_qkv[2 * D:3 * D][qc], (128, DL)).copy()
        wp_np = np.ascontiguousarray(
            w_proj[g * DL:(g + 1) * DL, :].reshape(PAIRS, 128, D).transpose(1, 0, 2)
        )
        xb = x[min(b, x.shape[0] - 1), :N] if x.ndim == 3 else x[:N]
        in_maps.append({
            "x": np.ascontiguousarray(xb).astype(np.float16),
            "wqk": wqk_np.astype(np.float16),
            "wv": wv_np.astype(np.float16),
            "bqk": bqk_np,
            "bv": bv_np,
            "wproj": wp_np.astype(np.float16),
        })
    return in_maps


_NC_CACHE = {}


def kernel(x, w_qkv, b_qkv, w_proj, b_proj):
    global LAST_EXEC_NS
    x = np.asarray(x, dtype=np.float32)
    b_proj = np.asarray(b_proj, dtype=np.float32)
    if N_FULL not in _NC_CACHE:
        _NC_CACHE[N_FULL] = build(N_FULL)
    nc = _NC_CACHE[N_FULL]
    in_maps = shard_inputs(x, w_qkv, b_qkv, w_proj)
    trace = os.environ.get("KERNEL_TRACE", "0") == "1"
    res = run_bass_kernel_spmd(
        nc, in_maps, core_ids=list(range(NCORES)), trace=trace,
        trace_cores=[0] if trace else None,
    )
    LAST_EXEC_NS = res.exec_time_ns
    full = np.empty((B, N_FULL, D), np.float32)
    for b in range(B):
        r0, r1 = res.results[2 * b], res.results[2 * b + 1]
        full[b] = r0["out"].astype(np.float32) + r1["out"].astype(np.float32)
        full[b][3 * 512:] += (r0["out2"].astype(np.float32)
                              + r1["out2"].astype(np.float32))
    full += b_proj[None, None, :]
    return full


# revision 9
# speedup vs baseline: 1.3062x; 1.0045x over previous
"""Multi-head attention block (B=4, N=2048, D=1024, H=16) on 8 trn2 NeuronCores.

Sharding: core c -> (batch b = c//2, head-group g = c%2). Each core computes
attention for 8 heads of one batch plus the partial output projection over its
512 head-dims; the host sums the two partials per batch and adds b_proj.

Per-core kernel (fp16 data path, fp32 PSUM accumulation):
  1. x is cast to fp16 on the host; xT tiles arrive via XBAR DMA transpose
     straight from DRAM (no PE transposes, no PSUM->SBUF copies).
  2. qT/kT computed head-transposed ([dims, tokens], lhsT = w slice),
     bias-added into fp16; v computed natural ([tokens, dims]) with a ones
     column per (k-tile, head) group (v_aug) so the flipped PV matmul also
     yields the softmax denominator.
  3. S^T tiles [k=128, q=512] for the two heads of a pair via two
     row-group-packed matmuls into one 2-bank PSUM tile [128, 1024].
  4. E = exp(scale * S^T) on ScalarE straight out of PSUM into fp16.
  5. PV flipped: out[q, d] per (head, 128-query block): lhsT = E slice
     [128k, 128q], rhs = v_aug slice [128k, 65]; 65-column matmuls
     accumulate over k-tiles. PV for k-tile j is emitted after S^T of
     k-tile j+1 so the in-order PE queue never stalls on the exp.
  6. Tail: 2 reciprocals + 2 broadcast multiplies normalize into fp16 u
     tiles; XBAR DMA transpose writes them back as [dims, tokens] for the
     fp16 projection.
"""

import os
import sys

import numpy as np

try:
    import concourse.bass as bass
except ImportError:  # harness runs from a bare directory
    sys.path.insert(0, "/opt/trn_rl_repo")
    import concourse.bass as bass

import concourse.mybir as mybir
import concourse.tile as tile
from concourse.bass_utils import run_bass_kernel_spmd
from concourse.masks import make_identity

F32 = mybir.dt.float32
F16 = mybir.dt.float16
EXP = mybir.ActivationFunctionType.Exp
ADD = mybir.AluOpType.add
MULT = mybir.AluOpType.mult

B, N_FULL, D = 4, 2048, 1024
H, HD = 16, 64
NCORES = 8
GROUPS = 2          # head-groups (tensor parallel)
HL = H // GROUPS    # 8 heads per core
DL = HL * HD        # 512 local head-dims per core
PAIRS = HL // 2     # 4 head pairs
SCALE = HD ** -0.5
VG = HD + 1         # v dims + ones column per (k-tile, head)

LAST_EXEC_NS = None
EXPS = []  # debug: (p, qn, kt) per emitted exp, in ACT-stream order


def _split_multiwait_matmuls(raw: bytes) -> bytes:
    """This container's walrus allows at most one sync-wait per Matmult.

    Tile attaches up to 3. Hoist the extras onto standalone EventSemaphore
    instructions inserted immediately before the matmul on the same engine
    (identical semantics: the sequencer blocks on them in program order).
    """
    import json

    bir = json.loads(raw)
    n = [0]

    def fix_block(block):
        insts = block.get("instructions")
        if not isinstance(insts, list):
            return
        out = []
        for ins in insts:
            si = ins.get("sync_info") if isinstance(ins, dict) else None
            if (
                isinstance(ins, dict)
                and ins.get("opcode") != "EventSemaphore"
                and si
                and len(si.get("on_wait") or []) > 1
            ):
                waits = si["on_wait"]
                for w in waits[1:]:
                    n[0] += 1
                    out.append({
                        "debug": ins.get("debug", 0),
                        "engine": ins["engine"],
                        "ins": [],
                        "name": f"I-waitfix-{n[0]}",
                        "opcode": "EventSemaphore",
                        "outs": [],
                        "sync_info": {"on_update": [], "on_wait": [w]},
                    })
                si["on_wait"] = waits[:1]
            out.append(ins)
        block["instructions"] = out

    for fn in bir.get("functions", []):
        for block in fn.get("blocks", []):
            fix_block(block)
    return json.dumps(bir).encode()


def build(N=N_FULL):
    NK = N // 128   # k tiles of 128
    NQ = N // 512   # q tiles of 512
    NTT = N // 512  # token tiles of 512 for the qkv projection

    nc = bass.Bass("TRN2", target_bir_lowering=False)
    x = nc.dram_tensor("x", [N, D], F16, kind="ExternalInput")
    # [ii, otile(4 q-pairs then 4 k-pairs), io, 128] so each DMA slab is
    # contiguous per partition.
    wqk = nc.dram_tensor("wqk", [128, 8, 8, 128], F16, kind="ExternalInput")
    wv = nc.dram_tensor("wv", [128, 8, DL], F16, kind="ExternalInput")
    bqk = nc.dram_tensor("bqk", [128, 8], F32, kind="ExternalInput")
    bv = nc.dram_tensor("bv", [128, DL], F32, kind="ExternalInput")
    wproj = nc.dram_tensor("wproj", [128, PAIRS, D], F16, kind="ExternalInput")
    out = nc.dram_tensor("out", [N, D], F16, kind="ExternalOutput")
    # partial projection (pairs 0-1) of the last query tile; the host adds
    # it onto out[3*512:], letting most of the final proj leave the tail
    out2 = nc.dram_tensor("out2", [512, D], F16, kind="ExternalOutput")

    with tile.TileContext(nc) as tc:
        with (
            tc.tile_pool(name="const", bufs=1) as const_pool,
            tc.tile_pool(name="wres", bufs=1) as wres_pool,
            tc.tile_pool(name="wqs", bufs=3) as wqs_pool,
            tc.tile_pool(name="xt", bufs=4) as xt_pool,
            tc.tile_pool(name="qk", bufs=1) as qk_pool,
            tc.tile_pool(name="vg", bufs=1) as vg_pool,
            tc.tile_pool(name="at", bufs=2) as at_pool,
            tc.tile_pool(name="ep", bufs=4) as e_pool,
            tc.tile_pool(name="rp", bufs=2) as r_pool,
            tc.tile_pool(name="up", bufs=6) as u_pool,
            tc.tile_pool(name="sg", bufs=10) as sg_pool,
            tc.tile_pool(name="ob", bufs=3) as ob_pool,
            tc.tile_pool(name="psst", bufs=2, space="PSUM") as pss_pool,
            tc.tile_pool(name="pspv", bufs=2, space="PSUM") as psv_pool,
            tc.tile_pool(name="ps512", bufs=2, space="PSUM") as p5_pool,
        ):
            # Warm up the ACT exp table while the first DMAs are in flight so
            # the first real exp doesn't pay the table load.
            warm = const_pool.tile([128, 1], F32)
            nc.gpsimd.memset(warm[:, :], 0.0)
            nc.scalar.activation(warm[:, :], warm[:, :], EXP)

            bqk_sb = const_pool.tile([128, 8], F32)

            # fp16 identity for the PE transposes in the attention tail
            # (fp16 memset is ISA-invalid, so build in f32 and cast)
            ident32 = const_pool.tile([128, 128], F32)
            make_identity(nc, ident32[:, :])
            ident16 = const_pool.tile([128, 128], F16)
            nc.vector.tensor_copy(ident16[:, :], ident32[:, :])

            qT = qk_pool.tile([128, PAIRS, N], F16, tag="qT")
            kT = qk_pool.tile([128, PAIRS, N], F16, tag="kT")
            # Flat v layout: per (k-tile, head) a 65-column group = 64 v-dims
            # + ones column (PV denominator column after the flip).
            vaug = vg_pool.tile([128, NK * HL * VG], F16, tag="vaug")
            ones_view = vaug[:, :].rearrange(
                "p (g c) -> p g c", c=VG)[:, :, HD:HD + 1]
            nc.vector.tensor_scalar(
                out=ones_view, in0=warm[:, None, 0:1].broadcast_to(
                    [128, NK * HL, 1]),
                scalar1=0.0, scalar2=1.0, op0=MULT, op1=ADD,
            )

            class Chain:
                """One (pair, qn) attention chain, PV pipelined one kt back.

                Can be spilled mid-way: the PSUM partials move to SBUF
                segment tiles so another chain can use the PSUM banks, and
                segments are merged back in finish().
                """

                def __init__(self, p, qn):
                    self.p, self.qn = p, qn
                    self.pvA = self.pvB = None
                    self.segs = [None, None]
                    self.prev = None   # (e2, kt) awaiting its PV
                    self.first = True

                def _pvs(self):
                    return ((0, self.pvA), (1, self.pvB))

                def _pv(self):
                    e2, kt = self.prev
                    self.prev = None
                    for hh, pv in self._pvs():
                        vo = (kt * HL + 2 * self.p + hh) * VG
                        for qs in range(4):
                            nc.tensor.matmul(
                                pv[:, qs, :],
                                lhsT=e2[:, hh * 512 + qs * 128:
                                        hh * 512 + (qs + 1) * 128],
                                rhs=vaug[:, vo:vo + VG],
                                start=(self.first and qs == 0),
                                stop=False,
                                skip_group_check=True,
                            )
                        if hh == 1:
                            self.first = False

                def step(self, kt):
                    if self.pvA is None:
                        self.pvA = psv_pool.tile([128, 4, VG], F32, tag="pv",
                                                 name="pvA")
                        self.pvB = psv_pool.tile([128, 4, VG], F32, tag="pv",
                                                 name="pvB")
                        self.first = True
                    q0 = self.qn * 512
                    k0 = kt * 128
                    stab = pss_pool.tile([128, 1024], F32, tag="st",
                                         name="stab")
                    for fo, base in ((0, 0), (512, 64)):
                        nc.tensor.matmul(
                            stab[:, fo:fo + 512],
                            lhsT=kT[base:base + 64, self.p, k0:k0 + 128],
                            rhs=qT[base:base + 64, self.p, q0:q0 + 512],
                            start=True,
                            stop=True,
                            tile_position=(base, 0),
                            skip_group_check=True,
                        )
                    e2 = e_pool.tile([128, 1024], F16, tag="e", name="e2")
                    nc.scalar.activation(e2[:, :], stab[:, :], EXP, scale=SCALE)
                    EXPS.append((self.p, self.qn, kt))
                    if self.prev is not None:
                        self._pv()
                    self.prev = (e2, kt)

                def spill(self):
                    """Drain the pending PV and move partials to SBUF."""
                    if self.prev is not None:
                        self._pv()
                    for hh, pv in self._pvs():
                        if self.segs[hh] is None:
                            seg = sg_pool.tile([128, 4, VG], F32, tag="sg",
                                               name="seg")
                            nc.vector.tensor_copy(seg[:, :, :], pv[:, :, :])
                        else:
                            seg = sg_pool.tile([128, 4, VG], F32, tag="sg",
                                               name="seg")
                            nc.vector.tensor_tensor(
                                out=seg[:, :, :], in0=pv[:, :, :],
                                in1=self.segs[hh][:, :, :], op=ADD)
                        self.segs[hh] = seg
                    self.pvA = self.pvB = None

                def finish(self, at_t):
                    if self.prev is not None:
                        self._pv()
                    p = self.p
                    # merge spilled segments, then normalize by the
                    # per-query denominator (column 64) during the fp16 copy
                    rcs = r_pool.tile([128, 2, 4, 1], F32, tag="rc", name="rcs")
                    srcs = []
                    for hh, pv in self._pvs():
                        if self.segs[hh] is not None:
                            fin = sg_pool.tile([128, 4, VG], F32, tag="sg",
                                               name="fin")
                            nc.vector.tensor_tensor(
                                out=fin[:, :, :], in0=pv[:, :, :],
                                in1=self.segs[hh][:, :, :], op=ADD)
                            srcs.append(fin)
                        else:
                            srcs.append(pv)
                    for hh, src in enumerate(srcs):
                        nc.vector.reciprocal(
                            rcs[:, hh, :, :], src[:, :, HD:HD + 1])
                    tr = p5_pool.tile([128, 512], F32, tag="p512", name="tr")
                    for hh, src in enumerate(srcs):
                        u = u_pool.tile([128, 4, HD], F16, tag="u", name="u")
                        nc.vector.tensor_tensor(
                            out=u[:, :, :],
                            in0=src[:, :, 0:HD],
                            in1=rcs[:, hh, :, :].broadcast_to([128, 4, HD]),
                            op=MULT,
                        )
                        for qs in range(4):
                            nc.tensor.matmul(
                                tr[hh * 64:(hh + 1) * 64,
                                   qs * 128:(qs + 1) * 128],
                                lhsT=u[:, qs, :],
                                rhs=ident16[:, :],
                                start=True,
                                stop=True,
                                skip_group_check=True,
                            )
                    nc.vector.tensor_copy(at_t[:, p, :], tr[:, :])
                    self.pvA = self.pvB = None
                    self.segs = [None, None]

            # Early chain: (pair 0, qn 0) runs during the kv phase — its
            # k-tiles become valid t-tile by t-tile, so its exps fill the
            # otherwise ACT-idle prefix.
            at0 = at_pool.tile([128, PAIRS, 512], F16, tag="at", name="at0")

            xts = []

            def qk_quarter(ti, o, state, q, xt):
                """Two ics of a q/k projection block, for side-work
                pumping: keeps per-pump PE cost at ~0.43us."""
                if q == 0:
                    wo = wqs_pool.tile([128, 8, 128], F16, tag="wo")
                    nc.sync.dma_start(wo[:, :, :], wqk[:, o, :, :])
                    qp = p5_pool.tile([128, 512], F32, tag="p512", name="qp")
                    state[:] = [wo, qp]
                wo, qp = state
                for ic in range(q * 2, q * 2 + 2):
                    nc.tensor.matmul(
                        qp[:, :],
                        lhsT=wo[:, ic, :],
                        rhs=xt[:, ic, :],
                        start=(ic == 0),
                        stop=(ic == 7),
                    )
                if q == 3:
                    dst = qT if o < 4 else kT
                    nc.vector.tensor_scalar_add(
                        dst[:, o % 4, ti * 512:(ti + 1) * 512], qp[:, :],
                        bqk_sb[:, o:o + 1],
                    )

            def qk_side(ti, o):
                state = []
                return [lambda q=q: qk_quarter(ti, o, state, q, xts[ti])
                        for q in (0, 1, 2, 3)]

            def qk_group(ti, o, xt):
                """One 128-dim output block of the q/k projection."""
                wo = wqs_pool.tile([128, 8, 128], F16, tag="wo")
                nc.sync.dma_start(wo[:, :, :], wqk[:, o, :, :])
                qp = p5_pool.tile([128, 512], F32, tag="p512", name="qp")
                for ic in range(8):
                    nc.tensor.matmul(
                        qp[:, :],
                        lhsT=wo[:, ic, :],
                        rhs=xt[:, ic, :],
                        start=(ic == 0),
                        stop=(ic == 7),
                    )
                dst = qT if o < 4 else kT
                nc.vector.tensor_scalar_add(
                    dst[:, o % 4, ti * 512:(ti + 1) * 512], qp[:, :],
                    bqk_sb[:, o:o + 1],
                )

            # ---- phase 1: xT (DMA transpose from DRAM), v, kT for every
            # token tile, plus the qn0 queries; q projections for qn1..3 are
            # deferred into the ACT-bound attention stretch (JIT q). The qn0
            # chains run here against the PE-dense stretch, spilling their
            # PSUM partials so only one chain holds banks at a time.
            chains0 = [Chain(p, 0) for p in range(PAIRS)]
            chA, chB, chC, chD = chains0
            ch10 = Chain(0, 1)

            def v_group(ti, s, xt):
                r = ti * 4 + s
                vp = p5_pool.tile([128, DL], F32, tag="p512", name="vp")
                for ic in range(8):
                    nc.tensor.matmul(
                        vp[:, :],
                        lhsT=xt[:, ic, s * 128:(s + 1) * 128],
                        rhs=wv_sb[:, ic, :],
                        start=(ic == 0),
                        stop=(ic == 7),
                    )
                nc.vector.tensor_tensor(
                    out=vaug[:, r * HL * VG:(r + 1) * HL * VG].rearrange(
                        "p (h c) -> p h c", c=VG)[:, :, 0:HD],
                    in0=vp[:, :].rearrange("p (h d) -> p h d", h=HL),
                    in1=bv_sb[:, :].rearrange("p (h d) -> p h d", h=HL),
                    op=ADD,
                )

            for ti in range(NTT):
                xt = xt_pool.tile([128, 8, 512], F16, tag="xt",
                                  name=f"xt{ti}")
                xts.append(xt)
                if ti == 0:
                    # wo tiles first, then quarter-tile transposes: the first
                    # q/k matmuls chase each 128-token chunk so PE work is
                    # absorbed into the DMA-bound prefix
                    wos0 = {}
                    for o in (4, 0):
                        wo_pre = wqs_pool.tile([128, 8, 128], F16, tag="wo",
                                               name=f"wo_pre{o}")
                        nc.sync.dma_start(wo_pre[:, :, :], wqk[:, o, :, :])
                        wos0[o] = wo_pre
                    for c in range(4):
                        nc.sync.dma_start_transpose(
                            xt[:, :, c * 128:(c + 1) * 128],
                            x[c * 128:(c + 1) * 128, :])
                    nc.sync.dma_start(bqk_sb[:, :], bqk[:, :])
                else:
                    nc.sync.dma_start_transpose(
                        xt[:, :, :], x[ti * 512:(ti + 1) * 512, :])
                if ti == 0:
                    # queries/keys for pair 0 first so the exp stream starts
                    # as early as possible; chain B follows A inside ti0.
                    qps = {o: p5_pool.tile([128, 512], F32, tag="p512",
                                            name=f"qp{o}") for o in (4, 0)}
                    for c in range(4):
                        for o in (4, 0):
                            for ic in range(8):
                                nc.tensor.matmul(
                                    qps[o][:, c * 128:(c + 1) * 128],
                                    lhsT=wos0[o][:, ic, :],
                                    rhs=xt[:, ic, c * 128:(c + 1) * 128],
                                    start=(c == 0 and ic == 0),
                                    stop=(c == 3 and ic == 7),
                                    skip_group_check=True,
                                )
                    for o in (4, 0):
                        dst = qT if o < 4 else kT
                        nc.vector.tensor_scalar_add(
                            dst[:, o % 4, 0:512], qps[o][:, :],
                            bqk_sb[:, o:o + 1])
                    bv_sb = const_pool.tile([128, DL], F32)
                    nc.sync.dma_start(bv_sb[:, :], bv[:, :])
                    wv_sb = wres_pool.tile([128, 8, DL], F16)
                    nc.sync.dma_start(wv_sb[:, :, :], wv[:, :, :])
                    chA.step(0)
                    v_group(0, 0, xt)
                    chA.step(1)
                    qk_group(0, 5, xt)
                    v_group(0, 1, xt)
                    chA.step(2)
                    qk_group(0, 1, xt)
                    v_group(0, 2, xt)
                    chA.step(3)
                    v_group(0, 3, xt)
                    chB.step(0)
                    chA.spill()
                    chB.step(1)
                    chB.step(2)
                    chB.step(3)
                    chB.spill()
                    for o in (6, 2, 7, 3):
                        qk_group(0, o, xt)
                elif ti == 1:
                    qk_group(1, 6, xt)
                    chC.step(0)
                    chC.step(1)
                    v_group(1, 0, xt)
                    chC.step(2)
                    v_group(1, 1, xt)
                    chC.step(3)
                    v_group(1, 2, xt)
                    chC.step(4)
                    v_group(1, 3, xt)
                    chC.step(5)
                    chC.step(6)
                    chC.step(7)
                    qk_group(1, 7, xt)
                    chD.step(0)
                    chC.spill()
                    for kt in range(1, 8):
                        chD.step(kt)
                    chD.spill()
                    qk_group(1, 4, xt)
                    qk_group(1, 5, xt)
                elif ti == 2:
                    qk_group(2, 4, xt)
                    chA.step(4)
                    chA.step(5)
                    v_group(2, 0, xt)
                    chA.step(6)
                    v_group(2, 1, xt)
                    chA.step(7)
                    v_group(2, 2, xt)
                    chA.step(8)
                    v_group(2, 3, xt)
                    chA.step(9)
                    chA.step(10)
                    chA.step(11)
                    qk_group(2, 5, xt)
                    chB.step(4)
                    chA.spill()
                    for kt in range(5, 8):
                        chB.step(kt)
                    qk_group(2, 6, xt)
                    for kt in range(8, 12):
                        chB.step(kt)
                    chB.spill()
                    qk_group(2, 7, xt)
                else:
                    qk_group(3, 6, xt)
                    chC.step(8)
                    chC.step(9)
                    v_group(3, 0, xt)
                    chC.step(10)
                    v_group(3, 1, xt)
                    chC.step(11)
                    v_group(3, 2, xt)
                    chC.step(12)
                    v_group(3, 3, xt)
                    chC.step(13)
                    chC.step(14)
                    chC.step(15)
                    qk_group(3, 7, xt)
                    chD.step(8)
                    chC.finish(at0)
                    for kt in range(9, 16):
                        chD.step(kt)
                    qk_group(3, 4, xt)
                    chA.step(12)
                    qk_group(3, 5, xt)
                    chD.finish(at0)
                    chA.step(13)
                    chA.step(14)
                    chA.step(15)
                    chB.step(12)
                    chA.finish(at0)
                    chB.step(13)
                    chB.step(14)
                    qk_group(1, 0, xts[1])
                    chB.step(15)
                    chB.finish(at0)
                    # first k-tiles of the next query tile's first chain:
                    # phase 2 is exp-bound, so every exp pulled into the
                    # PE-bound kv phase is nearly free
                    for kt in range(3):
                        ch10.step(kt)

            # w_proj is first read by the projection, deep into the
            # attention phase; loading it here keeps the head-of-queue DMA
            # slots for the x tiles the qkv matmuls are waiting on.
            wp_sb = wres_pool.tile([128, PAIRS, D], F16)
            nc.sync.dma_start(wp_sb[:, :, :], wproj[:, :, :])

            def proj(at_t, qn_t):
                for s in range(4):
                    t0 = qn_t * 512 + s * 128
                    for e in range(2):
                        op_ = p5_pool.tile([128, 512], F32, tag="p512", name="op")
                        for p_ in range(PAIRS):
                            nc.tensor.matmul(
                                op_[:, :],
                                lhsT=at_t[:, p_, s * 128:(s + 1) * 128],
                                rhs=wp_sb[:, p_, e * 512:(e + 1) * 512],
                                start=(p_ == 0),
                                stop=(p_ == PAIRS - 1),
                            )
                        ob = ob_pool.tile([128, 512], F16, tag="ob")
                        nc.vector.tensor_copy(ob[:, :], op_[:, :])
                        nc.sync.dma_start(
                            out[t0:t0 + 128, e * 512:(e + 1) * 512], ob[:, :])

            # ---- attention + projection, one 512-query tile at a time.
            # The stretch is ACT(exp)-bound; JIT q projections for qn+1,
            # proj(qn-1) matmul groups, and output copies are queued as side
            # work and pumped one item per odd k-tile into the PE-idle slack.
            side = []

            def pump():
                if side:
                    side.pop(0)()

            def proj_part(at_t, t0, s, e, state, half, dst=None):
                if half == 0:
                    state[:] = [p5_pool.tile([128, 512], F32, tag="p512",
                                             name="op")]
                op_ = state[0]
                for p_ in (half * 2, half * 2 + 1):
                    nc.tensor.matmul(
                        op_[:, :],
                        lhsT=at_t[:, p_, s * 128:(s + 1) * 128],
                        rhs=wp_sb[:, p_, e * 512:(e + 1) * 512],
                        start=(p_ == 0),
                        stop=(p_ == PAIRS - 1),
                    )
                if half == 1:
                    ob = ob_pool.tile([128, 512], F16, tag="ob")
                    nc.vector.tensor_copy(ob[:, :], op_[:, :])
                    d = out if dst is None else dst
                    nc.sync.dma_start(
                        d[t0:t0 + 128, e * 512:(e + 1) * 512], ob[:, :])

            def proj_side(at_t, qn_t):
                work = []
                for s in range(4):
                    t0 = qn_t * 512 + s * 128
                    for e in range(2):
                        state = []
                        work.extend(
                            lambda s=s, e=e, t0=t0, st=state, h=h:
                            proj_part(at_t, t0, s, e, st, h)
                            for h in (0, 1))
                return work

            def proj_group(at_t, t0, s, e, pairs=range(PAIRS), dst=None):
                op_ = p5_pool.tile([128, 512], F32, tag="p512", name="op")
                pl = list(pairs)
                for p_ in pl:
                    nc.tensor.matmul(
                        op_[:, :],
                        lhsT=at_t[:, p_, s * 128:(s + 1) * 128],
                        rhs=wp_sb[:, p_, e * 512:(e + 1) * 512],
                        start=(p_ == pl[0]),
                        stop=(p_ == pl[-1]),
                    )
                ob = ob_pool.tile([128, 512], F16, tag="ob")
                nc.vector.tensor_copy(ob[:, :], op_[:, :])
                d = out if dst is None else dst
                nc.sync.dma_start(
                    d[t0:t0 + 128, e * 512:(e + 1) * 512], ob[:, :])

            # Flat phase-2 plan over qn1..3; each chain's finish is
            # deferred until two steps into the next chain so the exp stream
            # never breaks at a chain boundary. JIT q projections and proj
            # output groups ride the side-work queue, pumped one item per
            # odd k-tile into the ACT-bound stretch's PE-idle slack.
            plan = []
            ats = {0: at0}
            for qn in range(1, NQ):
                ats[qn] = at_pool.tile([128, PAIRS, 512], F16, tag="at",
                                       name="at")
                plan += [((ch10, range(3, NK), 1) if (qn == 1 and p == 0)
                          else (Chain(p, qn), range(NK), qn))
                         for p in range(PAIRS)]

            for o in (1, 2, 3):
                side.extend(qk_side(1, o))
            side.extend(qk_side(2, 0))
            side.extend(proj_side(ats[0], 0))

            deferred = None
            for ch, kts, qn in plan:
                if qn == 2 and ch.p == 0:
                    for o in (1, 2, 3):
                        side.extend(qk_side(2, o))
                    side.extend(qk_side(3, 0))
                    side.extend(proj_side(ats[1], 1))
                if qn == 3 and ch.p == 0:
                    for o in (1, 2, 3):
                        side.extend(qk_side(3, o))
                    side.extend(proj_side(ats[2], 2))
                if qn == 3 and ch.p == 2:
                    side.extend(
                        lambda s=s, e=e: proj_group(
                            ats[3], s * 128, s, e, pairs=range(2), dst=out2)
                        for s in range(4) for e in range(2))
                    # (each item is a 2-matmul group + copy: ~0.43us PE)
                for idx, kt in enumerate(kts):
                    ch.step(kt)
                    if idx == 1 and deferred is not None:
                        dch, dqn = deferred
                        dch.finish(ats[dqn])
                        deferred = None
                    if idx >= 5:
                        pump()
                deferred = (ch, qn)
            dch, dqn = deferred
            dch.finish(ats[dqn])
            while side:
                pump()
            for s in range(4):
                for e in range(2):
                    op_ = p5_pool.tile([128, 512], F32, tag="p512", name="op")
                    for p_ in (2, 3):
                        nc.tensor.matmul(
                            op_[:, :],
                            lhsT=ats[3][:, p_, s * 128:(s + 1) * 128],
                            rhs=wp_sb[:, p_, e * 512:(e + 1) * 512],
                            start=(p_ == 2),
                            stop=(p_ == 3),
                        )
                    ob = ob_pool.tile([128, 512], F16, tag="obt",
                                      name="obt", bufs=4)
                    if e == 0:
                        nc.vector.tensor_copy(ob[:, :], op_[:, :])
                        eng = nc.sync
                    else:
                        nc.scalar.activation(
                            ob[:, :], op_[:, :],
                            mybir.ActivationFunctionType.Copy)
                        eng = nc.scalar
                    eng.dma_start(
                        out[3 * 512 + s * 128:3 * 512 + s * 128 + 128,
                            e * 512:(e + 1) * 512], ob[:, :])
    _orig_to_json = nc.to_json_bytes
    nc.to_json_bytes = lambda: _split_multiwait_matmuls(_orig_to_json())
    return nc


def shard_inputs(x, w_qkv, b_qkv, w_proj, N=N_FULL):
    """Build the 8 per-core input maps from full inputs."""
    x = np.ascontiguousarray(np.asarray(x, dtype=np.float32))
    w_qkv = np.asarray(w_qkv, dtype=np.float32)
    b_qkv = np.asarray(b_qkv, dtype=np.float32)
    w_proj = np.asarray(w_proj, dtype=np.float32)
    in_maps = []
    for c in range(NCORES):
        b, g = divmod(c, 2)
        qc = slice(g * DL, (g + 1) * DL)
        wq = w_qkv[:, 0 * D:1 * D][:, qc]
        wk = w_qkv[:, 1 * D:2 * D][:, qc]
        wv_ = w_qkv[:, 2 * D:3 * D][:, qc]
        wqk_np = np.empty((128, 8, 8, 128), np.float32)
        bqk_np = np.empty((128, 8), np.float32)
        for o in range(8):
            src = wq if o < 4 else wk
            bsrc = b_qkv[0:D][qc] if o < 4 else b_qkv[D:2 * D][qc]
            blk = src[:, (o % 4) * 128:(o % 4 + 1) * 128].reshape(8, 128, 128)
            wqk_np[:, o] = blk.transpose(1, 0, 2)
            bqk_np[:, o] = bsrc[(o % 4) * 128:(o % 4 + 1) * 128]
        wv_np = np.ascontiguousarray(wv_.reshape(8, 128, DL).transpose(1, 0, 2))
        bv_np = np.broadcast_to(b_qkv[2 * D:3 * D][qc], (128, DL)).copy()
        wp_np = np.ascontiguousarray(
            w_proj[g * DL:(g + 1) * DL, :].reshape(PAIRS, 128, D).transpose(1, 0, 2)
        )
        xb = x[min(b, x.shape[0] - 1), :N] if x.ndim == 3 else x[:N]
        in_maps.append({
            "x": np.ascontiguousarray(xb).astype(np.float16),
            "wqk": wqk_np.astype(np.float16),
            "wv": wv_np.astype(np.float16),
            "bqk": bqk_np,
            "bv": bv_np,
            "wproj": wp_np.astype(np.float16),
        })
    return in_maps


_NC_CACHE = {}


def kernel(x, w_qkv, b_qkv, w_proj, b_proj):
    global LAST_EXEC_NS
    x = np.asarray(x, dtype=np.float32)
    b_proj = np.asarray(b_proj, dtype=np.float32)
    if N_FULL not in _NC_CACHE:
        _NC_CACHE[N_FULL] = build(N_FULL)
    nc = _NC_CACHE[N_FULL]
    in_maps = shard_inputs(x, w_qkv, b_qkv, w_proj)
    trace = os.environ.get("KERNEL_TRACE", "0") == "1"
    res = run_bass_kernel_spmd(
        nc, in_maps, core_ids=list(range(NCORES)), trace=trace,
        trace_cores=[0] if trace else None,
    )
    LAST_EXEC_NS = res.exec_time_ns
    full = np.empty((B, N_FULL, D), np.float32)
    for b in range(B):
        r0, r1 = res.results[2 * b], res.results[2 * b + 1]
        full[b] = r0["out"].astype(np.float32) + r1["out"].astype(np.float32)
        full[b][3 * 512:] += (r0["out2"].astype(np.float32)
                              + r1["out2"].astype(np.float32))
    full += b_proj[None, None, :]
    return full
